# revision 5
# baseline (speedup 1.0000x reference)
"""Trainium2 Bass kernel for nn_Adapter (moe_routing).

Reference computation (per router m in [0,12), batch b in [0,32)):
    e = expert_index[m, b]
    z = x[b] @ down_w[m, e] + down_b[m, e]     # [S, D]
    z = z * sigmoid(z)                          # SiLU
    u[m, b] = z @ up_w[m, e]                    # [S, C]

Strategy:
  - Data-parallel over batch B=32 across 8 cores (4 batches per core).
  - Expert routing (the gather over expert_index) is done on HOST: each
    core receives the already-gathered per-(m,b) weight tables, laid out
    exactly as the SBUF tiles want them, pre-cast to bf16.
  - Device, per (b, m-pair): routers are processed two at a time packed
    into the 128x128 PE array:
      * down-proj: z^T[D=64,S] for m0 -> PE cols 0-63, m1 -> cols 64-127
        (col tiling), accumulating over 8 K-chunks of C=1024.
      * SiLU+bias on the combined [128,S] PSUM tile in one ScalarE op,
        output bf16.
      * up-proj: m0 -> PE rows 0-63, m1 -> rows 64-127 (row tiling),
        K=D=64, interleaved so consecutive instructions overlap in the
        array.
    PSUM evicted to SBUF as bf16 by ScalarE/VectorE (alternating);
    two fully contiguous 512KiB DMAs out per (m,b) in [p, sc, c]
    layout which the host unpermutes.
  - Output returned to host as bf16, host upcasts to f32 and stitches.

Schedule (variant 2, from HW trace analysis):
  - Startup: the framework preamble ends ~7-9us; the first down matmul
    needs only wd[b0,pair0] (256KB) + the first xt k-chunks, so those
    ride the sync HWDGE queue (idle until outputs start) k-chunked,
    with xt split into a [128,2,S] head tile and a [128,6,S] rest tile.
    First matmul ~13.8us (was 21.8us with one 1MB xt DMA behind the
    bulk weights on a single queue).
  - Bulk input loads are emitted upfront (all batches) on the gpsimd
    SWDGE queue so transfers order strictly by consumption; bulk weight
    tables move as single 1.25MB DMAs (PAIR_CHUNK=5) - per-pair loads
    measurably lose to the ~0.7us/DMA SWDGE issue overhead. Routing
    batch 1 via the scalar HWDGE queue (KERNEL_MULTIQ=1) measured
    neutral; DVE cannot issue DMAs at all.
  - Pool depths (KERNEL_BUFS=aggr2, default): wpool 4-deep so every
    batch's weights stream during the 10-45us ramp window where HBM has
    slack (instead of b2/b3 deferring to 40-85us and stealing bandwidth
    from the output stream mid-kernel), u-tiles 5-deep (10MB) because
    the frontloaded input burst starves the early OUT stream and a 4-deep
    u pool fills and stalls the PE ~5-6us mid-kernel, and b0's bulk
    weights ride sync ahead of any output DMA (removes the 13-25us
    input-wait stalls of b0's j>=1). Paired A/B: aggr2 beat aggr
    (4-deep u) by 8-16us, which beat shallow prefetch by 1-9us.
  - Tail: per-sc output DMAs + two-engine split evictions for the last
    router pairs drain the final outputs faster.
  - Middle: PE-paced. PE stream content is 294912 cycles = 123us at
    2.4GHz (up-proj K=64 uses half the array; that is a hard floor for
    this dataflow - out_elems/128 per cycle). The HAM power limiter
    grants ~45-126us of full clock (thermal-state dependent) then
    duty-cycles to 50%, adding 10-30us run-to-run. PSUM eviction
    (25.2MB/core f32->bf16) is balanced scalar/vector at ~4.4/4.9us
    per router pair vs 5.1us of PE; gpsimd cannot read PSUM on TRN2.

Perf model: ~52 GFLOP as bf16 on 8 cores; per core 67.25MB of HBM
traffic at the measured ~420GB/s sustained = 160us transfer floor
(+~10us startup). Measured 185-190us typical cold, 203-233us when the
HAM limiter has been heated by immediately preceding runs (the
duty-cycle grant shrinks). Negative results, measured: fp8e4m3
DoubleRow down-proj (cost model promises 0.5 cyc/row; HW runs it at
~1x and throttles harder -> 239us), per-pair weight DMAs (issue-rate
bound), gpsimd as third eviction engine (PSUM access rejected by BIR
verifier).
"""

import os
import sys
import time

sys.path.insert(0, "/opt/trn_rl_repo")

import numpy as np
import ml_dtypes

M, N_EXP, C, D = 12, 8, 1024, 64
B, S = 32, 512
NCORES = 8
BL = B // NCORES          # batches per core = 4
KC = C // 128             # contraction chunks for down-proj = 8
SC = S // 128             # output row chunks for up-proj = 4
JP = M // 2               # router pairs per batch = 6

BF16 = ml_dtypes.bfloat16

# granularity of the bulk weight-table DMAs in router pairs (1 = per-pair,
# 5 = one bulk DMA). Measured: 5 wins — SWDGE descriptor-gen issue rate
# (~0.7us/DMA on gpsimd) costs more than the finer dependencies save.
PAIR_CHUNK = int(os.environ.get("KERNEL_PAIR_CHUNK", "5"))
# spread input DMA issue across the scalar HWDGE queue for batch 1 to
# break the single-queue issue-rate limit early on. A/B measured no
# reliable win over gpsimd-only (chip-state noise ±20us dominates; best
# observed run was with 0), so default off.
MULTIQ = int(os.environ.get("KERNEL_MULTIQ", "0"))
# tile-pool depth preset, see _build_v2
BUFS = os.environ.get("KERNEL_BUFS", "aggr2")

# set by test.py to collect the profile
TRACE = bool(os.environ.get("KERNEL_TRACE"))
last_results = None

_nc_cache = {}


def _ensure_ntff_hook():
    """The agent image's `antenv` lacks `axon_hooks`, so the boot-time NTFF
    profile hook registration degrades silently and bass_utils' trace path
    crashes on import. Shim the module and install the ctypes hook."""
    import types

    if "antenv.axon_hooks" in sys.modules:
        return
    mod = types.ModuleType("antenv.axon_hooks")
    store = [None]
    mod.set_axon_ntff_profile_hook = lambda h: store.__setitem__(0, h)
    mod.get_axon_ntff_profile_hook = lambda: store[0]
    sys.modules["antenv.axon_hooks"] = mod
    try:
        import antenv

        antenv.axon_hooks = mod
    except ImportError:
        pass
    try:
        from trn_agent_boot.trn_boot import _ntff_profile_via_ctypes

        so_path = "/opt/axon/libaxon_pjrt.so"
        if os.path.exists(so_path):
            hook = _ntff_profile_via_ctypes(so_path)
            if hook is not None:
                mod.set_axon_ntff_profile_hook(hook)
    except Exception:
        pass


_ensure_ntff_hook()


def _build(variant=0):
    import concourse.mybir as mybir
    from concourse import bacc, tile

    bf16 = mybir.dt.bfloat16
    f32 = mybir.dt.float32
    AF = mybir.ActivationFunctionType

    if variant == 2:
        return _build_v2()

    nc = bacc.Bacc(
        "TRN2",
        target_bir_lowering=False,
        debug=False,
        num_devices=NCORES,
        num_swdge_queues=4,
    )
    xt_d = nc.declare_dram_parameter("xt", [BL, 128, KC, S], bf16, isOutput=False)
    wd_d = nc.declare_dram_parameter("wd", [BL, 128, JP, KC, 128], bf16, isOutput=False)
    wu_d = nc.declare_dram_parameter("wu", [BL, 128, JP, C], bf16, isOutput=False)
    bias_d = nc.declare_dram_parameter("bias", [BL, 128, JP], f32, isOutput=False)
    # [m, b, p, sc, c]: per-(m,b) output is fully contiguous; host unpermutes
    out_d = nc.declare_dram_parameter("out", [M, BL, 128, SC, C], bf16, isOutput=True)

    with tile.TileContext(nc) as tc:
        with (
            tc.tile_pool(name="xin", bufs=2) as xin_pool,
            tc.tile_pool(name="wpool", bufs=2) as w_pool,
            tc.tile_pool(name="zt", bufs=3) as zt_pool,
            tc.tile_pool(name="usb", bufs=4) as u_pool,
            tc.tile_pool(name="pz", bufs=2, space="PSUM") as pz_pool,
            tc.tile_pool(name="pu", bufs=3, space="PSUM") as pu_pool,
        ):
            for b in range(BL):
                xt_sb = xin_pool.tile([128, KC, S], bf16, tag="xt")
                if variant == 1 and b == 0:
                    pass  # issued below, after the j0 weight chunks
                else:
                    nc.gpsimd.dma_start(xt_sb[:], xt_d[b])
                wd_sb = w_pool.tile([128, JP, KC, 128], bf16, tag="wd")
                wu_sb = w_pool.tile([128, JP, C], bf16, tag="wu")
                bias_sb = w_pool.tile([128, JP], f32, tag="bias")
                # first pair's weights land first so PE can start the
                # batch's j=0 matmuls without waiting for the full tables
                # (cuts the PE stall at batch boundaries that re-throttles
                # the HAM clock gate)
                eng0 = nc.sync if (variant == 1 and b == 0) else nc.gpsimd
                eng0.dma_start(wd_sb[:, 0:1], wd_d[b, :, 0:1])
                if variant == 1 and b == 0:
                    nc.sync.dma_start(xt_sb[:], xt_d[b])
                eng0.dma_start(wu_sb[:, 0], wu_d[b, :, 0])
                eng0.dma_start(bias_sb[:], bias_d[b])
                nc.gpsimd.dma_start(wd_sb[:, 1:JP], wd_d[b, :, 1:JP])
                nc.gpsimd.dma_start(wu_sb[:, 1:JP], wu_d[b, :, 1:JP])

                for j in range(JP):
                    m0, m1 = 2 * j, 2 * j + 1
                    # down-proj, col-packed: m0 -> PE cols 0-63 -> psum
                    # partitions 0-63; m1 -> cols 64-127.
                    # both routers' down-weights are stacked along the
                    # stationary free dim on host, so one full-array matmul
                    # per k-chunk computes the pair (half the instructions,
                    # FWL-eligible 128-col weight loads)
                    psum_z = pz_pool.tile([128, S], f32, tag="pz")
                    for k in range(KC):
                        nc.tensor.matmul(
                            psum_z[:],
                            lhsT=wd_sb[:, j, k, :],
                            rhs=xt_sb[:, k, :],
                            start=(k == 0),
                            stop=(k == KC - 1),
                        )
                    # SiLU(z + bias) for both routers in one op, cast to bf16
                    zt_sb = zt_pool.tile([128, S], bf16, tag="zt")
                    nc.scalar.activation(
                        zt_sb[:], psum_z[:], AF.Silu, bias=bias_sb[:, j : j + 1]
                    )
                    # up-proj, row-packed: m0 -> PE rows 0-63, m1 -> rows
                    # 64-127, interleaved so the array works on both at once.
                    u0 = u_pool.tile([128, SC, C], bf16, tag="u0")
                    u1 = u_pool.tile([128, SC, C], bf16, tag="u1")
                    ev = j % 2
                    for sc in range(SC):
                        p0 = pu_pool.tile([128, C], f32, tag="pu")
                        p1 = pu_pool.tile([128, C], f32, tag="pu")
                        for cc in range(2):
                            nc.tensor.matmul(
                                p0[:, cc * 512 : (cc + 1) * 512],
                                lhsT=zt_sb[0:64, sc * 128 : (sc + 1) * 128],
                                rhs=wu_sb[0:64, j, cc * 512 : (cc + 1) * 512],
                                start=True,
                                stop=True,
                                tile_position=(0, 0),
                            )
                            nc.tensor.matmul(
                                p1[:, cc * 512 : (cc + 1) * 512],
                                lhsT=zt_sb[64:128, sc * 128 : (sc + 1) * 128],
                                rhs=wu_sb[64:128, j, cc * 512 : (cc + 1) * 512],
                                start=True,
                                stop=True,
                                tile_position=(64, 0),
                            )
                        tail = b == BL - 1 and j >= JP - 2
                        for pt, ut in ((p0, u0), (p1, u1)):
                            dst = ut[:, sc, :]
                            if tail:
                                # kernel tail is eviction-chain-bound: split
                                # each eviction across both engines so the
                                # last tiles drain twice as fast
                                nc.scalar.copy(dst[:, 0:512], pt[:, 0:512])
                                nc.vector.tensor_copy(dst[:, 512:C], pt[:, 512:C])
                            elif ev % 2 == 0:
                                nc.scalar.copy(dst, pt[:])
                            else:
                                nc.vector.tensor_copy(dst, pt[:])
                            ev += 1
                    if variant == 1 and b == BL - 1 and j == JP - 1:
                        for sc in range(SC):
                            nc.sync.dma_start(out_d[m0, b, :, sc], u0[:, sc])
                            nc.sync.dma_start(out_d[m1, b, :, sc], u1[:, sc])
                    else:
                        for half in range(2):
                            hs = slice(half * 2, half * 2 + 2)
                            nc.sync.dma_start(out_d[m0, b, :, hs], u0[:, hs])
                            nc.sync.dma_start(out_d[m1, b, :, hs], u1[:, hs])
    nc.compile()
    return nc


def _build_v2():
    """Variant 1 middle structure (PSUM eviction split scalar/vector is
    already balanced; gpsimd is PSUM-blind on TRN2 so it cannot help),
    with startup and tail fixes from the HW trace:

    1. Startup (21.8us -> ~9us): the first matmul only needs wd[b0,pair0]
       and the first k-chunks of xt[b0]; load those first, k-chunked, on
       the sync HWDGE queue (idle until outputs start) so the PE starts
       as soon as ~400KB has landed instead of waiting for the full 1MB
       xt DMA queued behind the bulk weight DMAs.
    2. wpool bufs=3: input prefetch runs two batches ahead, smoothing
       HBM contention between the input stream and the output stream.
    3. Tail drain: per-sc output DMAs + two-engine evictions for the
       last router pair (from variant 1).
    """
    import concourse.mybir as mybir
    from concourse import bacc, tile

    bf16 = mybir.dt.bfloat16
    f32 = mybir.dt.float32
    AF = mybir.ActivationFunctionType

    nc = bacc.Bacc(
        "TRN2",
        target_bir_lowering=False,
        debug=False,
        num_devices=NCORES,
        num_swdge_queues=4,
    )
    xt_d = nc.declare_dram_parameter("xt", [BL, 128, KC, S], bf16, isOutput=False)
    wd_d = nc.declare_dram_parameter("wd", [BL, 128, JP, KC, 128], bf16, isOutput=False)
    wu_d = nc.declare_dram_parameter("wu", [BL, 128, JP, C], bf16, isOutput=False)
    bias_d = nc.declare_dram_parameter("bias", [BL, 128, JP], f32, isOutput=False)
    out_d = nc.declare_dram_parameter("out", [M, BL, 128, SC, C], bf16, isOutput=True)

    with tile.TileContext(nc) as tc:
        # KERNEL_BUFS presets (per-partition SBUF): aggr2 = frontload +
        # deep u (202KB) - the 4-deep-u aggr variant stalled the PE ~5us
        # when the early input burst starved the out stream and the 8MB
        # u-pool filled; safe = shallow prefetch. zt needs only 2.
        if BUFS == "aggr2":
            xin_b, w_b, zt_b, u_b = 3, 4, 2, 5
        elif BUFS == "aggr":
            xin_b, w_b, zt_b, u_b = 4, 4, 3, 4
        else:
            xin_b, w_b, zt_b, u_b = 2, 3, 3, 5
        with (
            tc.tile_pool(name="xin", bufs=xin_b) as xin_pool,
            tc.tile_pool(name="wpool", bufs=w_b) as w_pool,
            tc.tile_pool(name="zt", bufs=zt_b) as zt_pool,
            tc.tile_pool(name="usb", bufs=u_b) as u_pool,
            tc.tile_pool(name="pz", bufs=2, space="PSUM") as pz_pool,
            tc.tile_pool(name="pu", bufs=3, space="PSUM") as pu_pool,
        ):
            # --- input load emission, all batches upfront, so the DMA
            # issues sit at the HEAD of each issuing engine's program
            # order and transfers flow strictly in consumption order
            # (KERNEL_MULTIQ=1 additionally routes batch 1 via the scalar
            # HWDGE queue; measured neutral, default off).
            batch_tiles = []
            for b in range(BL):
                # xt split into two tiles so the first down matmuls of a
                # batch only depend on a 256KB transfer, not the full 1MB
                xt0_sb = xin_pool.tile([128, 2, S], bf16, tag="xt0")
                xtr_sb = xin_pool.tile([128, KC - 2, S], bf16, tag="xtr")
                wd_sb = w_pool.tile([128, JP, KC, 128], bf16, tag="wd")
                wu_sb = w_pool.tile([128, JP, C], bf16, tag="wu")
                bias_sb = w_pool.tile([128, JP], f32, tag="bias")
                batch_tiles.append((xt0_sb, xtr_sb, wd_sb, wu_sb, bias_sb))
                # HWDGE issue engines are only SP (sync, reserved for the
                # critical b0 loads + outputs) and Activation (scalar);
                # DVE cannot issue DMAs
                if MULTIQ and b == 1:
                    ld = nc.scalar
                else:
                    ld = nc.gpsimd
                if b == 0:
                    # critical path rides the (otherwise idle) sync HWDGE,
                    # k-chunked so the first down matmul starts early
                    nc.sync.dma_start(wd_sb[:, 0:1], wd_d[b, :, 0:1])
                    nc.sync.dma_start(xt0_sb[:], xt_d[b, :, 0:2])
                    for kk in range(0, KC - 2, 2):
                        nc.sync.dma_start(
                            xtr_sb[:, kk : kk + 2], xt_d[b, :, kk + 2 : kk + 4]
                        )
                    if BUFS == "aggr2":
                        # ALL of b0's loads ride sync, ahead of any output
                        # DMA: they land ~15us instead of ~17-22us behind
                        # the congested gpsimd queue, removing the early
                        # down/up stalls of b0's j>=1
                        nc.sync.dma_start(bias_sb[:], bias_d[b])
                        nc.sync.dma_start(wu_sb[:, 0], wu_d[b, :, 0])
                        nc.sync.dma_start(wd_sb[:, 1:JP], wd_d[b, :, 1:JP])
                        nc.sync.dma_start(wu_sb[:, 1:JP], wu_d[b, :, 1:JP])
                    else:
                        nc.gpsimd.dma_start(bias_sb[:], bias_d[b])
                        nc.gpsimd.dma_start(wu_sb[:, 0], wu_d[b, :, 0])
                else:
                    if b == 1 and BUFS == "aggr2":
                        # WAW gate: this poke into b1's wd tile reads b0's
                        # xt0, so every gpsimd DMA (strict FIFO behind this
                        # tile's write) waits ~11us until b0's critical
                        # loads land - b0 gets the full HBM bandwidth for
                        # its working set instead of a ~50% share
                        b0_xt0 = batch_tiles[0][0]
                        nc.gpsimd.tensor_copy(
                            wd_sb[:, 0, 0, 0:1], b0_xt0[:, 0, 0:1]
                        )
                    ld.dma_start(wd_sb[:, 0:1], wd_d[b, :, 0:1])
                    ld.dma_start(xt0_sb[:], xt_d[b, :, 0:2])
                    ld.dma_start(xtr_sb[:], xt_d[b, :, 2:KC])
                    ld.dma_start(bias_sb[:], bias_d[b])
                    ld.dma_start(wu_sb[:, 0], wu_d[b, :, 0])
                # bulk weight loads; PAIR_CHUNK controls granularity
                # (5 = single bulk DMA pair, measured best)
                if b == 0 and BUFS == "aggr2":
                    pass  # bulk already issued on sync above
                else:
                    pc_ = PAIR_CHUNK
                    for jj in range(1, JP, pc_):
                        je = min(jj + pc_, JP)
                        ld.dma_start(wd_sb[:, jj:je], wd_d[b, :, jj:je])
                        ld.dma_start(wu_sb[:, jj:je], wu_d[b, :, jj:je])

            for b in range(BL):
                xt0_sb, xtr_sb, wd_sb, wu_sb, bias_sb = batch_tiles[b]
                for j in range(JP):
                    m0, m1 = 2 * j, 2 * j + 1
                    psum_z = pz_pool.tile([128, S], f32, tag="pz")
                    for k in range(KC):
                        xsrc = xt0_sb[:, k, :] if k < 2 else xtr_sb[:, k - 2, :]
                        nc.tensor.matmul(
                            psum_z[:],
                            lhsT=wd_sb[:, j, k, :],
                            rhs=xsrc,
                            start=(k == 0),
                            stop=(k == KC - 1),
                        )
                    zt_sb = zt_pool.tile([128, S], bf16, tag="zt")
                    nc.scalar.activation(
                        zt_sb[:], psum_z[:], AF.Silu, bias=bias_sb[:, j : j + 1]
                    )
                    u0 = u_pool.tile([128, SC, C], bf16, tag="u0")
                    u1 = u_pool.tile([128, SC, C], bf16, tag="u1")
                    ev = j % 2
                    tail = b == BL - 1 and j >= JP - 2
                    for sc in range(SC):
                        p0 = pu_pool.tile([128, C], f32, tag="pu")
                        p1 = pu_pool.tile([128, C], f32, tag="pu")
                        for cc in range(2):
                            nc.tensor.matmul(
                                p0[:, cc * 512 : (cc + 1) * 512],
                                lhsT=zt_sb[0:64, sc * 128 : (sc + 1) * 128],
                                rhs=wu_sb[0:64, j, cc * 512 : (cc + 1) * 512],
                                start=True,
                                stop=True,
                                tile_position=(0, 0),
                            )
                            nc.tensor.matmul(
                                p1[:, cc * 512 : (cc + 1) * 512],
                                lhsT=zt_sb[64:128, sc * 128 : (sc + 1) * 128],
                                rhs=wu_sb[64:128, j, cc * 512 : (cc + 1) * 512],
                                start=True,
                                stop=True,
                                tile_position=(64, 0),
                            )
                        for pt, ut in ((p0, u0), (p1, u1)):
                            dst = ut[:, sc, :]
                            if tail:
                                # drain the kernel tail twice as fast by
                                # splitting each eviction across engines
                                nc.scalar.copy(dst[:, 0:512], pt[:, 0:512])
                                nc.vector.tensor_copy(dst[:, 512:C], pt[:, 512:C])
                            elif ev % 2 == 0:
                                nc.scalar.copy(dst, pt[:])
                            else:
                                nc.vector.tensor_copy(dst, pt[:])
                            ev += 1
                    if b == BL - 1 and j == JP - 1:
                        for sc in range(SC):
                            nc.sync.dma_start(out_d[m0, b, :, sc], u0[:, sc])
                            nc.sync.dma_start(out_d[m1, b, :, sc], u1[:, sc])
                    else:
                        for half in range(2):
                            hs = slice(half * 2, half * 2 + 2)
                            nc.sync.dma_start(out_d[m0, b, :, hs], u0[:, hs])
                            nc.sync.dma_start(out_d[m1, b, :, hs], u1[:, hs])
    nc.compile()
    return nc


def _build_v3():
    """v2 + fp8e4m3 DoubleRow down-projection.

    The PE streams 1 moving column/cycle in bf16; DoubleRow fp8 streams 2
    (two 128-row k-tiles contracted at once, 0.5 cycles/row).  The down
    GEMM contracts C=1024 = 4 k-tile pairs.  fp8 alone loses too much
    precision (~3.6%), so use 3-term error compensation:
        x @ W  ~=  x_hi@W_hi + x_lo@W_hi + x_hi@W_lo
    (x = x_hi + x_lo exactly to second order; residual x_lo@W_lo ~ 0.1%).
    12 DoubleRow matmuls x 256 cycles replace 8 bf16 matmuls x 512 cycles
    per router pair: down-proj PE time 4096 -> 3072 cycles, -10us/core
    total.  W is pre-scaled by 64 on host (w~N(0,0.01) would land in the
    fp8 denormal range); the silu activation un-scales via scale=1/64.
    DMA bytes are unchanged (hi+lo fp8 = 2 bytes/elem = bf16).
    """
    import concourse.mybir as mybir
    from concourse import bacc, tile

    bf16 = mybir.dt.bfloat16
    fp8 = mybir.dt.float8e4
    f32 = mybir.dt.float32
    AF = mybir.ActivationFunctionType
    DR = mybir.MatmulPerfMode.DoubleRow
    KCP = KC // 2  # k-tile pairs = 4

    nc = bacc.Bacc(
        "TRN2",
        target_bir_lowering=False,
        debug=False,
        num_devices=NCORES,
        num_swdge_queues=4,
    )
    xh_d = nc.declare_dram_parameter("xh", [BL, 128, KCP, 2, S], fp8, isOutput=False)
    xl_d = nc.declare_dram_parameter("xl", [BL, 128, KCP, 2, S], fp8, isOutput=False)
    wh_d = nc.declare_dram_parameter(
        "wh", [BL, 128, JP, KCP, 2, 128], fp8, isOutput=False
    )
    wl_d = nc.declare_dram_parameter(
        "wl", [BL, 128, JP, KCP, 2, 128], fp8, isOutput=False
    )
    wu_d = nc.declare_dram_parameter("wu", [BL, 128, JP, C], bf16, isOutput=False)
    bias_d = nc.declare_dram_parameter("bias", [BL, 128, JP], f32, isOutput=False)
    out_d = nc.declare_dram_parameter("out", [M, BL, 128, SC, C], bf16, isOutput=True)

    with tile.TileContext(nc) as tc:
        with (
            tc.tile_pool(name="xin", bufs=2) as xin_pool,
            tc.tile_pool(name="wpool", bufs=3) as w_pool,
            tc.tile_pool(name="zt", bufs=3) as zt_pool,
            tc.tile_pool(name="usb", bufs=4) as u_pool,
            tc.tile_pool(name="pz", bufs=2, space="PSUM") as pz_pool,
            tc.tile_pool(name="pu", bufs=3, space="PSUM") as pu_pool,
        ):
            for b in range(BL):
                xh0_sb = xin_pool.tile([128, 2, S], fp8, tag="xh0")
                xhr_sb = xin_pool.tile([128, KCP - 1, 2, S], fp8, tag="xhr")
                xl0_sb = xin_pool.tile([128, 2, S], fp8, tag="xl0")
                xlr_sb = xin_pool.tile([128, KCP - 1, 2, S], fp8, tag="xlr")
                wh_sb = w_pool.tile([128, JP, KCP, 2, 128], fp8, tag="wh")
                wl_sb = w_pool.tile([128, JP, KCP, 2, 128], fp8, tag="wl")
                wu_sb = w_pool.tile([128, JP, C], bf16, tag="wu")
                bias_sb = w_pool.tile([128, JP], f32, tag="bias")
                if b == 0:
                    # critical path (term-0 operands) on the sync HWDGE
                    nc.sync.dma_start(wh_sb[:, 0:1], wh_d[b, :, 0:1])
                    nc.sync.dma_start(xh0_sb[:], xh_d[b, :, 0])
                    nc.sync.dma_start(xhr_sb[:], xh_d[b, :, 1:KCP])
                    nc.sync.dma_start(xl0_sb[:], xl_d[b, :, 0])
                    nc.sync.dma_start(xlr_sb[:], xl_d[b, :, 1:KCP])
                    nc.sync.dma_start(wl_sb[:, 0:1], wl_d[b, :, 0:1])
                    nc.gpsimd.dma_start(bias_sb[:], bias_d[b])
                    nc.gpsimd.dma_start(wu_sb[:, 0], wu_d[b, :, 0])
                else:
                    nc.gpsimd.dma_start(wh_sb[:, 0:1], wh_d[b, :, 0:1])
                    nc.gpsimd.dma_start(xh0_sb[:], xh_d[b, :, 0])
                    nc.gpsimd.dma_start(xhr_sb[:], xh_d[b, :, 1:KCP])
                    nc.gpsimd.dma_start(xl0_sb[:], xl_d[b, :, 0])
                    nc.gpsimd.dma_start(xlr_sb[:], xl_d[b, :, 1:KCP])
                    nc.gpsimd.dma_start(bias_sb[:], bias_d[b])
                    nc.gpsimd.dma_start(wu_sb[:, 0], wu_d[b, :, 0])
                    nc.gpsimd.dma_start(wl_sb[:, 0:1], wl_d[b, :, 0:1])
                for jj in range(1, JP):
                    nc.gpsimd.dma_start(wh_sb[:, jj : jj + 1], wh_d[b, :, jj : jj + 1])
                    nc.gpsimd.dma_start(wl_sb[:, jj : jj + 1], wl_d[b, :, jj : jj + 1])
                    nc.gpsimd.dma_start(wu_sb[:, jj], wu_d[b, :, jj])

                for j in range(JP):
                    m0, m1 = 2 * j, 2 * j + 1
                    psum_z = pz_pool.tile([128, S], f32, tag="pz")
                    terms = (
                        (xh0_sb, xhr_sb, wh_sb),
                        (xl0_sb, xlr_sb, wh_sb),
                        (xh0_sb, xhr_sb, wl_sb),
                    )
                    nt = len(terms)
                    for t, (x0, xr, ws) in enumerate(terms):
                        for kp in range(KCP):
                            rhs = x0[:] if kp == 0 else xr[:, kp - 1]
                            nc.tensor.matmul(
                                psum_z[:],
                                lhsT=ws[:, j, kp],
                                rhs=rhs,
                                start=(t == 0 and kp == 0),
                                stop=(t == nt - 1 and kp == KCP - 1),
                                perf_mode=DR,
                            )
                    zt_sb = zt_pool.tile([128, S], bf16, tag="zt")
                    nc.scalar.activation(
                        zt_sb[:],
                        psum_z[:],
                        AF.Silu,
                        bias=bias_sb[:, j : j + 1],
                        scale=1.0 / 64.0,
                    )
                    u0 = u_pool.tile([128, SC, C], bf16, tag="u0")
                    u1 = u_pool.tile([128, SC, C], bf16, tag="u1")
                    ev = j % 2
                    tail = b == BL - 1 and j >= JP - 2
                    for sc in range(SC):
                        p0 = pu_pool.tile([128, C], f32, tag="pu")
                        p1 = pu_pool.tile([128, C], f32, tag="pu")
                        for cc in range(2):
                            nc.tensor.matmul(
                                p0[:, cc * 512 : (cc + 1) * 512],
                                lhsT=zt_sb[0:64, sc * 128 : (sc + 1) * 128],
                                rhs=wu_sb[0:64, j, cc * 512 : (cc + 1) * 512],
                                start=True,
                                stop=True,
                                tile_position=(0, 0),
                            )
                            nc.tensor.matmul(
                                p1[:, cc * 512 : (cc + 1) * 512],
                                lhsT=zt_sb[64:128, sc * 128 : (sc + 1) * 128],
                                rhs=wu_sb[64:128, j, cc * 512 : (cc + 1) * 512],
                                start=True,
                                stop=True,
                                tile_position=(64, 0),
                            )
                        for pt, ut in ((p0, u0), (p1, u1)):
                            dst = ut[:, sc, :]
                            if tail:
                                nc.scalar.copy(dst[:, 0:512], pt[:, 0:512])
                                nc.vector.tensor_copy(dst[:, 512:C], pt[:, 512:C])
                            elif ev % 2 == 0:
                                nc.scalar.copy(dst, pt[:])
                            else:
                                nc.vector.tensor_copy(dst, pt[:])
                            ev += 1
                    if b == BL - 1 and j == JP - 1:
                        for sc in range(SC):
                            nc.sync.dma_start(out_d[m0, b, :, sc], u0[:, sc])
                            nc.sync.dma_start(out_d[m1, b, :, sc], u1[:, sc])
                    else:
                        for half in range(2):
                            hs = slice(half * 2, half * 2 + 2)
                            nc.sync.dma_start(out_d[m0, b, :, hs], u0[:, hs])
                            nc.sync.dma_start(out_d[m1, b, :, hs], u1[:, hs])
    nc.compile()
    return nc


def _build_v4():
    """v2 reworked around the HW-trace finding that the output stream is
    DMA-ISSUE-RATE bound, not bandwidth bound:

    Trace evidence (cold 196.7us / hot 210.6us spans): each DIRECT2D
    issue costs ~0.61us of sequencer time regardless of transfer size
    (128-row descriptor). v2 issues 96 x 512KB output DMAs on the single
    sync HWDGE queue = ~58us of issue time, which caps the output stream
    at ~420GB/s cold and HALF that when the HAM power limiter duty-cycles
    the sequencer clock - the 16 DMA engines starve (busy% drops 82->77)
    and the backlog drains in a 19-32us post-PE tail.

    Changes:
      1. One 1MB DMA per router (48 total): out_d[m, b] is already a
         contiguous [128, SC, C] region. u0's DMA rides the scalar HWDGE
         queue, u1's the sync queue -> ~14.4us of issue per queue, 4x
         slack vs HAM-throttled issue rate.
      2. Evictions pinned: p0 -> u0 always via scalar, p1 -> u1 always
         via vector (same 4+4 balance), so each queue's DMA trigger
         directly follows its own engine's final eviction of that tile.
      3. Startup split across both HWDGE queues: sync carries wd0, xt
         k-chunks, wd[1:2], wd[2:6]; scalar carries bias, wu[0:1],
         wu[1:6]. v2 serialized all 9 b0 issues on sync (~5.5us) and the
         j1 down-proj waited on the trailing 1.25MB wd[1:6] bulk DMA
         (5.5us PE gap at ~10.4us). Predicted: wd1 lands ~11us, PE
         steady from ~13us.
    """
    import concourse.mybir as mybir
    from concourse import bacc, tile

    bf16 = mybir.dt.bfloat16
    f32 = mybir.dt.float32
    AF = mybir.ActivationFunctionType

    nc = bacc.Bacc(
        "TRN2",
        target_bir_lowering=False,
        debug=False,
        num_devices=NCORES,
        num_swdge_queues=4,
    )
    xt_d = nc.declare_dram_parameter("xt", [BL, 128, KC, S], bf16, isOutput=False)
    wd_d = nc.declare_dram_parameter("wd", [BL, 128, JP, KC, 128], bf16, isOutput=False)
    wu_d = nc.declare_dram_parameter("wu", [BL, 128, JP, C], bf16, isOutput=False)
    bias_d = nc.declare_dram_parameter("bias", [BL, 128, JP], f32, isOutput=False)
    out_d = nc.declare_dram_parameter("out", [M, BL, 128, SC, C], bf16, isOutput=True)

    with tile.TileContext(nc) as tc:
        with (
            tc.tile_pool(name="xin", bufs=3) as xin_pool,
            tc.tile_pool(name="wpool", bufs=4) as w_pool,
            tc.tile_pool(name="zt", bufs=2) as zt_pool,
            tc.tile_pool(name="usb", bufs=5) as u_pool,
            tc.tile_pool(name="pz", bufs=2, space="PSUM") as pz_pool,
            tc.tile_pool(name="pu", bufs=3, space="PSUM") as pu_pool,
        ):
            batch_tiles = []
            for b in range(BL):
                xt0_sb = xin_pool.tile([128, 2, S], bf16, tag="xt0")
                xtr_sb = xin_pool.tile([128, KC - 2, S], bf16, tag="xtr")
                wd_sb = w_pool.tile([128, JP, KC, 128], bf16, tag="wd")
                wu_sb = w_pool.tile([128, JP, C], bf16, tag="wu")
                bias_sb = w_pool.tile([128, JP], f32, tag="bias")
                batch_tiles.append((xt0_sb, xtr_sb, wd_sb, wu_sb, bias_sb))
                if b == 0:
                    # b0 critical path split across BOTH HWDGE queues
                    nc.sync.dma_start(wd_sb[:, 0:1], wd_d[b, :, 0:1])
                    nc.sync.dma_start(xt0_sb[:], xt_d[b, :, 0:2])
                    nc.scalar.dma_start(bias_sb[:], bias_d[b])
                    nc.scalar.dma_start(wu_sb[:, 0:1], wu_d[b, :, 0:1])
                    for kk in range(0, KC - 2, 2):
                        nc.sync.dma_start(
                            xtr_sb[:, kk : kk + 2], xt_d[b, :, kk + 2 : kk + 4]
                        )
                    nc.sync.dma_start(wd_sb[:, 1:2], wd_d[b, :, 1:2])
                    nc.scalar.dma_start(wu_sb[:, 1:JP], wu_d[b, :, 1:JP])
                    nc.sync.dma_start(wd_sb[:, 2:JP], wd_d[b, :, 2:JP])
                else:
                    if b == 1:
                        # WAW gate: this poke into b1's wd tile reads b0's
                        # xt0, so every gpsimd DMA (strict FIFO behind this
                        # tile's write) waits until b0's critical loads
                        # land - b0 gets full HBM bandwidth for its
                        # working set instead of a ~50% share
                        b0_xt0 = batch_tiles[0][0]
                        nc.gpsimd.tensor_copy(wd_sb[:, 0, 0, 0:1], b0_xt0[:, 0, 0:1])
                    nc.gpsimd.dma_start(wd_sb[:, 0:1], wd_d[b, :, 0:1])
                    nc.gpsimd.dma_start(xt0_sb[:], xt_d[b, :, 0:2])
                    nc.gpsimd.dma_start(xtr_sb[:], xt_d[b, :, 2:KC])
                    nc.gpsimd.dma_start(bias_sb[:], bias_d[b])
                    nc.gpsimd.dma_start(wu_sb[:, 0], wu_d[b, :, 0])
                    nc.gpsimd.dma_start(wd_sb[:, 1:JP], wd_d[b, :, 1:JP])
                    nc.gpsimd.dma_start(wu_sb[:, 1:JP], wu_d[b, :, 1:JP])

            for b in range(BL):
                xt0_sb, xtr_sb, wd_sb, wu_sb, bias_sb = batch_tiles[b]
                for j in range(JP):
                    m0, m1 = 2 * j, 2 * j + 1
                    psum_z = pz_pool.tile([128, S], f32, tag="pz")
                    for k in range(KC):
                        xsrc = xt0_sb[:, k, :] if k < 2 else xtr_sb[:, k - 2, :]
                        nc.tensor.matmul(
                            psum_z[:],
                            lhsT=wd_sb[:, j, k, :],
                            rhs=xsrc,
                            start=(k == 0),
                            stop=(k == KC - 1),
                        )
                    zt_sb = zt_pool.tile([128, S], bf16, tag="zt")
                    nc.scalar.activation(
                        zt_sb[:], psum_z[:], AF.Silu, bias=bias_sb[:, j : j + 1]
                    )
                    u0 = u_pool.tile([128, SC, C], bf16, tag="u0")
                    u1 = u_pool.tile([128, SC, C], bf16, tag="u1")
                    for sc in range(SC):
                        p0 = pu_pool.tile([128, C], f32, tag="pu")
                        p1 = pu_pool.tile([128, C], f32, tag="pu")
                        for cc in range(2):
                            nc.tensor.matmul(
                                p0[:, cc * 512 : (cc + 1) * 512],
                                lhsT=zt_sb[0:64, sc * 128 : (sc + 1) * 128],
                                rhs=wu_sb[0:64, j, cc * 512 : (cc + 1) * 512],
                                start=True,
                                stop=True,
                                tile_position=(0, 0),
                            )
                            nc.tensor.matmul(
                                p1[:, cc * 512 : (cc + 1) * 512],
                                lhsT=zt_sb[64:128, sc * 128 : (sc + 1) * 128],
                                rhs=wu_sb[64:128, j, cc * 512 : (cc + 1) * 512],
                                start=True,
                                stop=True,
                                tile_position=(64, 0),
                            )
                        # pinned: scalar always evicts p0->u0, vector p1->u1,
                        # so each output queue's DMA follows its own engine
                        nc.scalar.copy(u0[:, sc, :], p0[:])
                        nc.vector.tensor_copy(u1[:, sc, :], p1[:])
                    nc.scalar.dma_start(out_d[m0, b], u0[:])
                    nc.sync.dma_start(out_d[m1, b], u1[:])
    nc.compile()
    return nc


def _build_v5():
    """v2 with EXACTLY one change: per-router 1MB output DMAs (48 on sync)
    instead of per-half 512KB ones (96) mid-kernel; tail unchanged.

    Rationale from the HW trace: each DIRECT2D issue costs ~0.61us of
    sync-sequencer time regardless of size (128-row descriptors), so v2
    spends ~58us issuing outputs - at the HAM-throttled (half-clock)
    issue rate that caps the output stream below HBM bandwidth and the
    backlog drains in a 19-32us post-PE tail. 48 issues halve that.
    (v4's further step - u0 DMAs on the scalar queue + pinned evictions +
    dual-queue startup - regressed to 252us: scalar-queue output DMAs
    stall the scalar engine's silu/eviction stream and the dual-queue
    startup scrambles b0's load ordering; avoided here.)
    """
    import concourse.mybir as mybir
    from concourse import bacc, tile

    bf16 = mybir.dt.bfloat16
    f32 = mybir.dt.float32
    AF = mybir.ActivationFunctionType

    nc = bacc.Bacc(
        "TRN2",
        target_bir_lowering=False,
        debug=False,
        num_devices=NCORES,
        num_swdge_queues=4,
    )
    xt_d = nc.declare_dram_parameter("xt", [BL, 128, KC, S], bf16, isOutput=False)
    wd_d = nc.declare_dram_parameter("wd", [BL, 128, JP, KC, 128], bf16, isOutput=False)
    wu_d = nc.declare_dram_parameter("wu", [BL, 128, JP, C], bf16, isOutput=False)
    bias_d = nc.declare_dram_parameter("bias", [BL, 128, JP], f32, isOutput=False)
    out_d = nc.declare_dram_parameter("out", [M, BL, 128, SC, C], bf16, isOutput=True)

    with tile.TileContext(nc) as tc:
        xin_b, w_b, zt_b, u_b = 3, 4, 2, 5
        with (
            tc.tile_pool(name="xin", bufs=xin_b) as xin_pool,
            tc.tile_pool(name="wpool", bufs=w_b) as w_pool,
            tc.tile_pool(name="zt", bufs=zt_b) as zt_pool,
            tc.tile_pool(name="usb", bufs=u_b) as u_pool,
            tc.tile_pool(name="pz", bufs=2, space="PSUM") as pz_pool,
            tc.tile_pool(name="pu", bufs=3, space="PSUM") as pu_pool,
        ):
            batch_tiles = []
            for b in range(BL):
                xt0_sb = xin_pool.tile([128, 2, S], bf16, tag="xt0")
                xtr_sb = xin_pool.tile([128, KC - 2, S], bf16, tag="xtr")
                wd_sb = w_pool.tile([128, JP, KC, 128], bf16, tag="wd")
                wu_sb = w_pool.tile([128, JP, C], bf16, tag="wu")
                bias_sb = w_pool.tile([128, JP], f32, tag="bias")
                batch_tiles.append((xt0_sb, xtr_sb, wd_sb, wu_sb, bias_sb))
                if b == 0:
                    nc.sync.dma_start(wd_sb[:, 0:1], wd_d[b, :, 0:1])
                    nc.sync.dma_start(xt0_sb[:], xt_d[b, :, 0:2])
                    for kk in range(0, KC - 2, 2):
                        nc.sync.dma_start(
                            xtr_sb[:, kk : kk + 2], xt_d[b, :, kk + 2 : kk + 4]
                        )
                    nc.sync.dma_start(bias_sb[:], bias_d[b])
                    nc.sync.dma_start(wu_sb[:, 0], wu_d[b, :, 0])
                    nc.sync.dma_start(wd_sb[:, 1:JP], wd_d[b, :, 1:JP])
                    nc.sync.dma_start(wu_sb[:, 1:JP], wu_d[b, :, 1:JP])
                else:
                    if b == 1:
                        # WAW gate: poke into b1's wd tile reads b0's xt0 so
                        # all gpsimd bulk DMAs queue behind b0's critical path
                        b0_xt0 = batch_tiles[0][0]
                        nc.gpsimd.tensor_copy(wd_sb[:, 0, 0, 0:1], b0_xt0[:, 0, 0:1])
                    nc.gpsimd.dma_start(wd_sb[:, 0:1], wd_d[b, :, 0:1])
                    nc.gpsimd.dma_start(xt0_sb[:], xt_d[b, :, 0:2])
                    nc.gpsimd.dma_start(xtr_sb[:], xt_d[b, :, 2:KC])
                    nc.gpsimd.dma_start(bias_sb[:], bias_d[b])
                    nc.gpsimd.dma_start(wu_sb[:, 0], wu_d[b, :, 0])
                    nc.gpsimd.dma_start(wd_sb[:, 1:JP], wd_d[b, :, 1:JP])
                    nc.gpsimd.dma_start(wu_sb[:, 1:JP], wu_d[b, :, 1:JP])

            for b in range(BL):
                xt0_sb, xtr_sb, wd_sb, wu_sb, bias_sb = batch_tiles[b]
                for j in range(JP):
                    m0, m1 = 2 * j, 2 * j + 1
                    psum_z = pz_pool.tile([128, S], f32, tag="pz")
                    for k in range(KC):
                        xsrc = xt0_sb[:, k, :] if k < 2 else xtr_sb[:, k - 2, :]
                        nc.tensor.matmul(
                            psum_z[:],
                            lhsT=wd_sb[:, j, k, :],
                            rhs=xsrc,
                            start=(k == 0),
                            stop=(k == KC - 1),
                        )
                    zt_sb = zt_pool.tile([128, S], bf16, tag="zt")
                    nc.scalar.activation(
                        zt_sb[:], psum_z[:], AF.Silu, bias=bias_sb[:, j : j + 1]
                    )
                    u0 = u_pool.tile([128, SC, C], bf16, tag="u0")
                    u1 = u_pool.tile([128, SC, C], bf16, tag="u1")
                    ev = j % 2
                    tail = b == BL - 1 and j >= JP - 2
                    for sc in range(SC):
                        p0 = pu_pool.tile([128, C], f32, tag="pu")
                        p1 = pu_pool.tile([128, C], f32, tag="pu")
                        for cc in range(2):
                            nc.tensor.matmul(
                                p0[:, cc * 512 : (cc + 1) * 512],
                                lhsT=zt_sb[0:64, sc * 128 : (sc + 1) * 128],
                                rhs=wu_sb[0:64, j, cc * 512 : (cc + 1) * 512],
                                start=True,
                                stop=True,
                                tile_position=(0, 0),
                            )
                            nc.tensor.matmul(
                                p1[:, cc * 512 : (cc + 1) * 512],
                                lhsT=zt_sb[64:128, sc * 128 : (sc + 1) * 128],
                                rhs=wu_sb[64:128, j, cc * 512 : (cc + 1) * 512],
                                start=True,
                                stop=True,
                                tile_position=(64, 0),
                            )
                        for pt, ut in ((p0, u0), (p1, u1)):
                            dst = ut[:, sc, :]
                            if tail:
                                nc.scalar.copy(dst[:, 0:512], pt[:, 0:512])
                                nc.vector.tensor_copy(dst[:, 512:C], pt[:, 512:C])
                            elif ev % 2 == 0:
                                nc.scalar.copy(dst, pt[:])
                            else:
                                nc.vector.tensor_copy(dst, pt[:])
                            ev += 1
                    if b == BL - 1 and j == JP - 1:
                        for sc in range(SC):
                            if V6_GP_OUT:
                                nc.gpsimd.dma_start(out_d[m0, b, :, sc], u0[:, sc])
                            else:
                                nc.sync.dma_start(out_d[m0, b, :, sc], u0[:, sc])
                            nc.sync.dma_start(out_d[m1, b, :, sc], u1[:, sc])
                    else:
                        # from b>=2 the gpsimd SWDGE queue has finished all
                        # input loads; routing u0's DMA there gives a second
                        # issue engine so sync's in-stream eviction waits no
                        # longer starve the 16 DMA engines (88-90% busy in
                        # the v5 trace, and the missing ~10% is the tail)
                        if V6_GP_OUT and b >= 2:
                            nc.gpsimd.dma_start(out_d[m0, b], u0[:])
                        else:
                            nc.sync.dma_start(out_d[m0, b], u0[:])
                        nc.sync.dma_start(out_d[m1, b], u1[:])
    nc.compile()
    return nc


V6_GP_OUT = False


def _build_v6():
    global V6_GP_OUT
    V6_GP_OUT = True
    try:
        return _build_v5()
    finally:
        V6_GP_OUT = False


_BUILDERS = {3: _build_v3, 4: _build_v4, 5: _build_v5, 6: _build_v6}


def _get_nc(variant=0):
    if variant not in _nc_cache:
        if variant in _BUILDERS:
            _nc_cache[variant] = _BUILDERS[variant]()
        else:
            _nc_cache[variant] = _build(variant)
    return _nc_cache[variant]


def kernel(x, expert_index, down_w, down_b, up_w):
    global last_results
    from concourse.bass_utils import run_bass_kernel_spmd

    x = np.asarray(x, dtype=np.float32)              # [B, S, C]
    idx = np.asarray(expert_index).astype(np.int64)  # [M, B]
    down_w = np.asarray(down_w, dtype=np.float32)    # [M, N, C, D]
    down_b = np.asarray(down_b, dtype=np.float32)    # [M, N, D]
    up_w = np.asarray(up_w, dtype=np.float32)        # [M, N, D, C]

    m_idx = np.arange(M)[:, None]
    wd_g = down_w[m_idx, idx]                        # [M, B, C, D]
    bb_g = down_b[m_idx, idx]                        # [M, B, D]
    wu_g = up_w[m_idx, idx]                          # [M, B, D, C]

    variant = int(os.environ.get("KERNEL_VARIANT", "4"))

    # xt[b, p, k, s] = x[b, s, k*128+p]
    xt_f = np.ascontiguousarray(
        x.transpose(0, 2, 1).reshape(B, KC, 128, S).transpose(0, 2, 1, 3)
    )
    # wd[b, p, j, k, dd]: dd in [0,128) holds router 2j (d=dd) in the low
    # 64 columns and router 2j+1 (d=dd-64) in the high 64 columns, so one
    # [128,128] stationary load covers the pair
    wd_f = np.ascontiguousarray(
        wd_g.reshape(JP, 2, B, KC, 128, D)
        .transpose(2, 4, 0, 3, 1, 5)
        .reshape(B, 128, JP, KC, 128)
    )
    # wu[b, p, j, c]: partitions 0-63 hold router 2j (d = p), partitions
    # 64-127 hold router 2j+1 (d = p-64)
    wu_p = wu_g.reshape(JP, 2, B, D, C).transpose(2, 1, 3, 0, 4)  # [B,2,D,JP,C]
    wu = np.ascontiguousarray(wu_p.reshape(B, 128, JP, C)).astype(BF16)
    # bias[b, p, j], same partition packing as wu
    bias_p = bb_g.reshape(JP, 2, B, D).transpose(2, 1, 3, 0)      # [B,2,D,JP]
    bias = np.ascontiguousarray(bias_p.reshape(B, 128, JP)).astype(np.float32)

    if variant == 3:
        F8 = ml_dtypes.float8_e4m3
        KCP = KC // 2
        xh = xt_f.astype(F8)
        xl = (xt_f - xh.astype(np.float32)).astype(F8)
        xh = xh.reshape(B, 128, KCP, 2, S)
        xl = xl.reshape(B, 128, KCP, 2, S)
        wds = wd_f * 64.0  # w ~ N(0, 0.01) sits in fp8 denormal range unscaled
        wh = wds.astype(F8)
        wl = (wds - wh.astype(np.float32)).astype(F8)
        wh = wh.reshape(B, 128, JP, KCP, 2, 128)
        wl = wl.reshape(B, 128, JP, KCP, 2, 128)
        per_core = {"xh": xh, "xl": xl, "wh": wh, "wl": wl, "wu": wu, "bias": bias}
    else:
        per_core = {
            "xt": xt_f.astype(BF16),
            "wd": wd_f.astype(BF16),
            "wu": wu,
            "bias": bias,
        }

    in_maps = []
    for core in range(NCORES):
        sl = slice(core * BL, (core + 1) * BL)
        in_maps.append({k: v[sl] for k, v in per_core.items()})

    nc = _get_nc(variant)
    trace_kwargs = {}
    if os.environ.get("KERNEL_TRACE_ALL"):
        trace_kwargs["trace_cores"] = list(range(NCORES))
    res = None
    for attempt in range(3):
        try:
            res = run_bass_kernel_spmd(
                nc, in_maps, core_ids=list(range(NCORES)), trace=TRACE, **trace_kwargs
            )
            break
        except Exception:
            # transient NRT_EXEC_UNIT_UNRECOVERABLE has been observed on a
            # process's first execute (stale device state from a prior
            # process); give the runtime a moment to recover, then retry
            if attempt == 2:
                raise
            time.sleep(10.0 * (attempt + 1))
    last_results = res

    out = np.empty((M, B, S, C), dtype=np.float32)
    for core in range(NCORES):
        sl = slice(core * BL, (core + 1) * BL)
        # dev out [M, BL, p, sc, c] -> [M, BL, s = sc*128+p, c]
        dev = res.results[core]["out"]
        out[:, sl] = dev.transpose(0, 1, 3, 2, 4).reshape(M, BL, S, C).astype(
            np.float32
        )
    return out



# revision 15
# speedup vs baseline: 1.1463x; 1.1463x over previous
"""Trainium2 Bass kernel for nn_Adapter (moe_routing).

Reference computation (per router m in [0,12), batch b in [0,32)):
    e = expert_index[m, b]
    z = x[b] @ down_w[m, e] + down_b[m, e]     # [S, D]
    z = z * sigmoid(z)                          # SiLU
    u[m, b] = z @ up_w[m, e]                    # [S, C]

Strategy:
  - Data-parallel over batch B=32 across 8 cores (4 batches per core).
  - Expert routing (the gather over expert_index) is done on HOST: each
    core receives the already-gathered per-(m,b) weight tables, laid out
    exactly as the SBUF tiles want them, pre-cast to bf16.
  - Device, per (b, m-pair): routers are processed two at a time packed
    into the 128x128 PE array:
      * down-proj: z^T[D=64,S] for m0 -> PE cols 0-63, m1 -> cols 64-127
        (col tiling), accumulating over 8 K-chunks of C=1024.
      * SiLU+bias on the combined [128,S] PSUM tile in one ScalarE op,
        output bf16.
      * up-proj: m0 -> PE rows 0-63, m1 -> rows 64-127 (row tiling),
        K=D=64, interleaved so consecutive instructions overlap in the
        array.
    PSUM evicted to SBUF as bf16 by ScalarE/VectorE (alternating);
    two fully contiguous 512KiB DMAs out per (m,b) in [p, sc, c]
    layout which the host unpermutes.
  - Output returned to host as bf16, host upcasts to f32 and stitches.

Schedule (variant 2, from HW trace analysis):
  - Startup: the framework preamble ends ~7-9us; the first down matmul
    needs only wd[b0,pair0] (256KB) + the first xt k-chunks, so those
    ride the sync HWDGE queue (idle until outputs start) k-chunked,
    with xt split into a [128,2,S] head tile and a [128,6,S] rest tile.
    First matmul ~13.8us (was 21.8us with one 1MB xt DMA behind the
    bulk weights on a single queue).
  - Bulk input loads are emitted upfront (all batches) on the gpsimd
    SWDGE queue so transfers order strictly by consumption; bulk weight
    tables move as single 1.25MB DMAs (PAIR_CHUNK=5) - per-pair loads
    measurably lose to the ~0.7us/DMA SWDGE issue overhead. Routing
    batch 1 via the scalar HWDGE queue (KERNEL_MULTIQ=1) measured
    neutral; DVE cannot issue DMAs at all.
  - Pool depths (KERNEL_BUFS=aggr2, default): wpool 4-deep so every
    batch's weights stream during the 10-45us ramp window where HBM has
    slack (instead of b2/b3 deferring to 40-85us and stealing bandwidth
    from the output stream mid-kernel), u-tiles 5-deep (10MB) because
    the frontloaded input burst starves the early OUT stream and a 4-deep
    u pool fills and stalls the PE ~5-6us mid-kernel, and b0's bulk
    weights ride sync ahead of any output DMA (removes the 13-25us
    input-wait stalls of b0's j>=1). Paired A/B: aggr2 beat aggr
    (4-deep u) by 8-16us, which beat shallow prefetch by 1-9us.
  - Tail: per-sc output DMAs + two-engine split evictions for the last
    router pairs drain the final outputs faster.
  - Middle: PE-paced. PE stream content is 294912 cycles = 123us at
    2.4GHz (up-proj K=64 uses half the array; that is a hard floor for
    this dataflow - out_elems/128 per cycle). The HAM power limiter
    grants ~45-126us of full clock (thermal-state dependent) then
    duty-cycles to 50%, adding 10-30us run-to-run. PSUM eviction
    (25.2MB/core f32->bf16) is balanced scalar/vector at ~4.4/4.9us
    per router pair vs 5.1us of PE; gpsimd cannot read PSUM on TRN2.

Perf model: ~52 GFLOP as bf16 on 8 cores; per core 67.12MB of HBM
traffic at the measured ~420GB/s sustained = 160us transfer floor
(+~10us startup). Measured 185-190us typical cold, 203-233us when the
HAM limiter has been heated by immediately preceding runs (the
duty-cycle grant shrinks). Negative results, measured: fp8e4m3
DoubleRow down-proj (cost model promises 0.5 cyc/row; HW runs it at
~1x and throttles harder -> 239us), per-pair weight DMAs (issue-rate
bound), gpsimd as third eviction engine (PSUM access rejected by BIR
verifier).

Session-2 findings (HW traces, paired A/B under shared thermal state):
  - The kernel is DMA-STREAM-bound end to end: 16 DMA queues at 82-90%
    busy over the whole span; PE (TensorMatrix) always finishes before
    the output backlog drains. Optimizing PE start time is worthless;
    keeping the DMA engines fed is everything.
  - Each DIRECT2D (HWDGE) issue costs ~0.61us of SEQUENCER time
    regardless of transfer size (observed on 512KB..1.25MB DMAs), and
    the HAM limiter halves the sequencer clock in its duty-cycle
    windows. v2's 96x512KB output DMAs on the single sync queue =
    ~58us of issue, marginal vs HBM rate, and the post-PE tail (19us
    cold / 32us hot) is the starved-issue backlog draining. v5 makes
    outputs one 1MB DMA per router (48 issues, out_d[m,b] contiguous):
    DMA busy% rose 82->88-90, tail 19->16.5 cold, best observed 178.3us
    (vs 181.8 baseline).
  - Tile deps are tracked per-TILE: a reader of wd_sb[:,0] waits for
    ALL of the tile's in-flight writers (the 1.25MB bulk DMA too).
    Also the scheduler MERGES a stream's semaphore waits (the first
    Tensor wait covers ~j0+j1's deps; fires at queue-count 8-9).
    Splitting wd/wu into {j0},{j1},{rest} tiles (v7) moves the first
    matmul 15.1->13.8us only, and delaying the gpsimd input flood
    (late_gate, v9) to protect b0's loads made HOT runs consistently
    WORSE (213-218 vs 209-210 paired): the flood delay pushes the whole
    input stream later, and the stream end time is what matters.
  - v4 (outputs on the scalar HWDGE queue + evictions pinned per
    engine + dual-queue b0 startup) collapsed to 252-260us: scalar's
    DMA issues/waits serialize with its silu+eviction stream and
    backpressure PSUM->PE; don't put output DMAs on a compute engine's
    queue.
  - Run-to-run HAM variance is +-25us and thermal state persists
    ACROSS test invocations (minutes); only paired interleaved A/B in
    one process (ab.py) gives comparable numbers.
"""

import os
import sys
import time

sys.path.insert(0, "/opt/trn_rl_repo")

import numpy as np
import ml_dtypes

M, N_EXP, C, D = 12, 8, 1024, 64
B, S = 32, 512
NCORES = 8
BL = B // NCORES          # batches per core = 4
KC = C // 128             # contraction chunks for down-proj = 8
SC = S // 128             # output row chunks for up-proj = 4
JP = M // 2               # router pairs per batch = 6

BF16 = ml_dtypes.bfloat16

# granularity of the bulk weight-table DMAs in router pairs (1 = per-pair,
# 5 = one bulk DMA). Measured: 5 wins — SWDGE descriptor-gen issue rate
# (~0.7us/DMA on gpsimd) costs more than the finer dependencies save.
PAIR_CHUNK = int(os.environ.get("KERNEL_PAIR_CHUNK", "5"))
# spread input DMA issue across the scalar HWDGE queue for batch 1 to
# break the single-queue issue-rate limit early on. A/B measured no
# reliable win over gpsimd-only (chip-state noise ±20us dominates; best
# observed run was with 0), so default off.
MULTIQ = int(os.environ.get("KERNEL_MULTIQ", "0"))
# tile-pool depth preset, see _build_v2
BUFS = os.environ.get("KERNEL_BUFS", "aggr2")

# set by test.py to collect the profile
TRACE = bool(os.environ.get("KERNEL_TRACE"))
last_results = None

_nc_cache = {}


def _ensure_ntff_hook():
    """The agent image's `antenv` lacks `axon_hooks`, so the boot-time NTFF
    profile hook registration degrades silently and bass_utils' trace path
    crashes on import. Shim the module and install the ctypes hook."""
    import types

    if "antenv.axon_hooks" in sys.modules:
        return
    mod = types.ModuleType("antenv.axon_hooks")
    store = [None]
    mod.set_axon_ntff_profile_hook = lambda h: store.__setitem__(0, h)
    mod.get_axon_ntff_profile_hook = lambda: store[0]
    sys.modules["antenv.axon_hooks"] = mod
    try:
        import antenv

        antenv.axon_hooks = mod
    except ImportError:
        pass
    try:
        from trn_agent_boot.trn_boot import _ntff_profile_via_ctypes

        so_path = "/opt/axon/libaxon_pjrt.so"
        if os.path.exists(so_path):
            hook = _ntff_profile_via_ctypes(so_path)
            if hook is not None:
                mod.set_axon_ntff_profile_hook(hook)
    except Exception:
        pass


_ensure_ntff_hook()


def _build(variant=0):
    import concourse.mybir as mybir
    from concourse import bacc, tile

    bf16 = mybir.dt.bfloat16
    f32 = mybir.dt.float32
    AF = mybir.ActivationFunctionType

    if variant == 2:
        return _build_v2()

    nc = bacc.Bacc(
        "TRN2",
        target_bir_lowering=False,
        debug=False,
        num_devices=NCORES,
        num_swdge_queues=4,
    )
    xt_d = nc.declare_dram_parameter("xt", [BL, 128, KC, S], bf16, isOutput=False)
    wd_d = nc.declare_dram_parameter("wd", [BL, 128, JP, KC, 128], bf16, isOutput=False)
    wu_d = nc.declare_dram_parameter("wu", [BL, 128, JP, C], bf16, isOutput=False)
    bias_d = nc.declare_dram_parameter("bias", [BL, 128, JP], f32, isOutput=False)
    # [m, b, p, sc, c]: per-(m,b) output is fully contiguous; host unpermutes
    out_d = nc.declare_dram_parameter("out", [M, BL, 128, SC, C], bf16, isOutput=True)

    with tile.TileContext(nc) as tc:
        with (
            tc.tile_pool(name="xin", bufs=2) as xin_pool,
            tc.tile_pool(name="wpool", bufs=2) as w_pool,
            tc.tile_pool(name="zt", bufs=3) as zt_pool,
            tc.tile_pool(name="usb", bufs=4) as u_pool,
            tc.tile_pool(name="pz", bufs=2, space="PSUM") as pz_pool,
            tc.tile_pool(name="pu", bufs=3, space="PSUM") as pu_pool,
        ):
            for b in range(BL):
                xt_sb = xin_pool.tile([128, KC, S], bf16, tag="xt")
                if variant == 1 and b == 0:
                    pass  # issued below, after the j0 weight chunks
                else:
                    nc.gpsimd.dma_start(xt_sb[:], xt_d[b])
                wd_sb = w_pool.tile([128, JP, KC, 128], bf16, tag="wd")
                wu_sb = w_pool.tile([128, JP, C], bf16, tag="wu")
                bias_sb = w_pool.tile([128, JP], f32, tag="bias")
                # first pair's weights land first so PE can start the
                # batch's j=0 matmuls without waiting for the full tables
                # (cuts the PE stall at batch boundaries that re-throttles
                # the HAM clock gate)
                eng0 = nc.sync if (variant == 1 and b == 0) else nc.gpsimd
                eng0.dma_start(wd_sb[:, 0:1], wd_d[b, :, 0:1])
                if variant == 1 and b == 0:
                    nc.sync.dma_start(xt_sb[:], xt_d[b])
                eng0.dma_start(wu_sb[:, 0], wu_d[b, :, 0])
                eng0.dma_start(bias_sb[:], bias_d[b])
                nc.gpsimd.dma_start(wd_sb[:, 1:JP], wd_d[b, :, 1:JP])
                nc.gpsimd.dma_start(wu_sb[:, 1:JP], wu_d[b, :, 1:JP])

                for j in range(JP):
                    m0, m1 = 2 * j, 2 * j + 1
                    # down-proj, col-packed: m0 -> PE cols 0-63 -> psum
                    # partitions 0-63; m1 -> cols 64-127.
                    # both routers' down-weights are stacked along the
                    # stationary free dim on host, so one full-array matmul
                    # per k-chunk computes the pair (half the instructions,
                    # FWL-eligible 128-col weight loads)
                    psum_z = pz_pool.tile([128, S], f32, tag="pz")
                    for k in range(KC):
                        nc.tensor.matmul(
                            psum_z[:],
                            lhsT=wd_sb[:, j, k, :],
                            rhs=xt_sb[:, k, :],
                            start=(k == 0),
                            stop=(k == KC - 1),
                        )
                    # SiLU(z + bias) for both routers in one op, cast to bf16
                    zt_sb = zt_pool.tile([128, S], bf16, tag="zt")
                    nc.scalar.activation(
                        zt_sb[:], psum_z[:], AF.Silu, bias=bias_sb[:, j : j + 1]
                    )
                    # up-proj, row-packed: m0 -> PE rows 0-63, m1 -> rows
                    # 64-127, interleaved so the array works on both at once.
                    u0 = u_pool.tile([128, SC, C], bf16, tag="u0")
                    u1 = u_pool.tile([128, SC, C], bf16, tag="u1")
                    ev = j % 2
                    for sc in range(SC):
                        p0 = pu_pool.tile([128, C], f32, tag="pu")
                        p1 = pu_pool.tile([128, C], f32, tag="pu")
                        for cc in range(2):
                            nc.tensor.matmul(
                                p0[:, cc * 512 : (cc + 1) * 512],
                                lhsT=zt_sb[0:64, sc * 128 : (sc + 1) * 128],
                                rhs=wu_sb[0:64, j, cc * 512 : (cc + 1) * 512],
                                start=True,
                                stop=True,
                                tile_position=(0, 0),
                            )
                            nc.tensor.matmul(
                                p1[:, cc * 512 : (cc + 1) * 512],
                                lhsT=zt_sb[64:128, sc * 128 : (sc + 1) * 128],
                                rhs=wu_sb[64:128, j, cc * 512 : (cc + 1) * 512],
                                start=True,
                                stop=True,
                                tile_position=(64, 0),
                            )
                        tail = b == BL - 1 and j >= JP - 2
                        for pt, ut in ((p0, u0), (p1, u1)):
                            dst = ut[:, sc, :]
                            if tail:
                                # kernel tail is eviction-chain-bound: split
                                # each eviction across both engines so the
                                # last tiles drain twice as fast
                                nc.scalar.copy(dst[:, 0:512], pt[:, 0:512])
                                nc.vector.tensor_copy(dst[:, 512:C], pt[:, 512:C])
                            elif ev % 2 == 0:
                                nc.scalar.copy(dst, pt[:])
                            else:
                                nc.vector.tensor_copy(dst, pt[:])
                            ev += 1
                    if variant == 1 and b == BL - 1 and j == JP - 1:
                        for sc in range(SC):
                            nc.sync.dma_start(out_d[m0, b, :, sc], u0[:, sc])
                            nc.sync.dma_start(out_d[m1, b, :, sc], u1[:, sc])
                    else:
                        for half in range(2):
                            hs = slice(half * 2, half * 2 + 2)
                            nc.sync.dma_start(out_d[m0, b, :, hs], u0[:, hs])
                            nc.sync.dma_start(out_d[m1, b, :, hs], u1[:, hs])
    nc.compile()
    return nc


def _build_v2():
    """Variant 1 middle structure (PSUM eviction split scalar/vector is
    already balanced; gpsimd is PSUM-blind on TRN2 so it cannot help),
    with startup and tail fixes from the HW trace:

    1. Startup (21.8us -> ~9us): the first matmul only needs wd[b0,pair0]
       and the first k-chunks of xt[b0]; load those first, k-chunked, on
       the sync HWDGE queue (idle until outputs start) so the PE starts
       as soon as ~400KB has landed instead of waiting for the full 1MB
       xt DMA queued behind the bulk weight DMAs.
    2. wpool bufs=3: input prefetch runs two batches ahead, smoothing
       HBM contention between the input stream and the output stream.
    3. Tail drain: per-sc output DMAs + two-engine evictions for the
       last router pair (from variant 1).
    """
    import concourse.mybir as mybir
    from concourse import bacc, tile

    bf16 = mybir.dt.bfloat16
    f32 = mybir.dt.float32
    AF = mybir.ActivationFunctionType

    nc = bacc.Bacc(
        "TRN2",
        target_bir_lowering=False,
        debug=False,
        num_devices=NCORES,
        num_swdge_queues=4,
    )
    xt_d = nc.declare_dram_parameter("xt", [BL, 128, KC, S], bf16, isOutput=False)
    wd_d = nc.declare_dram_parameter("wd", [BL, 128, JP, KC, 128], bf16, isOutput=False)
    wu_d = nc.declare_dram_parameter("wu", [BL, 128, JP, C], bf16, isOutput=False)
    bias_d = nc.declare_dram_parameter("bias", [BL, 128, JP], f32, isOutput=False)
    out_d = nc.declare_dram_parameter("out", [M, BL, 128, SC, C], bf16, isOutput=True)

    with tile.TileContext(nc) as tc:
        # KERNEL_BUFS presets (per-partition SBUF): aggr2 = frontload +
        # deep u (202KB) - the 4-deep-u aggr variant stalled the PE ~5us
        # when the early input burst starved the out stream and the 8MB
        # u-pool filled; safe = shallow prefetch. zt needs only 2.
        if BUFS == "aggr2":
            xin_b, w_b, zt_b, u_b = 3, 4, 2, 5
        elif BUFS == "aggr":
            xin_b, w_b, zt_b, u_b = 4, 4, 3, 4
        else:
            xin_b, w_b, zt_b, u_b = 2, 3, 3, 5
        with (
            tc.tile_pool(name="xin", bufs=xin_b) as xin_pool,
            tc.tile_pool(name="wpool", bufs=w_b) as w_pool,
            tc.tile_pool(name="zt", bufs=zt_b) as zt_pool,
            tc.tile_pool(name="usb", bufs=u_b) as u_pool,
            tc.tile_pool(name="pz", bufs=2, space="PSUM") as pz_pool,
            tc.tile_pool(name="pu", bufs=3, space="PSUM") as pu_pool,
        ):
            # --- input load emission, all batches upfront, so the DMA
            # issues sit at the HEAD of each issuing engine's program
            # order and transfers flow strictly in consumption order
            # (KERNEL_MULTIQ=1 additionally routes batch 1 via the scalar
            # HWDGE queue; measured neutral, default off).
            batch_tiles = []
            for b in range(BL):
                # xt split into two tiles so the first down matmuls of a
                # batch only depend on a 256KB transfer, not the full 1MB
                xt0_sb = xin_pool.tile([128, 2, S], bf16, tag="xt0")
                xtr_sb = xin_pool.tile([128, KC - 2, S], bf16, tag="xtr")
                wd_sb = w_pool.tile([128, JP, KC, 128], bf16, tag="wd")
                wu_sb = w_pool.tile([128, JP, C], bf16, tag="wu")
                bias_sb = w_pool.tile([128, JP], f32, tag="bias")
                batch_tiles.append((xt0_sb, xtr_sb, wd_sb, wu_sb, bias_sb))
                # HWDGE issue engines are only SP (sync, reserved for the
                # critical b0 loads + outputs) and Activation (scalar);
                # DVE cannot issue DMAs
                if MULTIQ and b == 1:
                    ld = nc.scalar
                else:
                    ld = nc.gpsimd
                if b == 0:
                    # critical path rides the (otherwise idle) sync HWDGE,
                    # k-chunked so the first down matmul starts early
                    nc.sync.dma_start(wd_sb[:, 0:1], wd_d[b, :, 0:1])
                    nc.sync.dma_start(xt0_sb[:], xt_d[b, :, 0:2])
                    for kk in range(0, KC - 2, 2):
                        nc.sync.dma_start(
                            xtr_sb[:, kk : kk + 2], xt_d[b, :, kk + 2 : kk + 4]
                        )
                    if BUFS == "aggr2":
                        # ALL of b0's loads ride sync, ahead of any output
                        # DMA: they land ~15us instead of ~17-22us behind
                        # the congested gpsimd queue, removing the early
                        # down/up stalls of b0's j>=1
                        nc.sync.dma_start(bias_sb[:], bias_d[b])
                        nc.sync.dma_start(wu_sb[:, 0], wu_d[b, :, 0])
                        nc.sync.dma_start(wd_sb[:, 1:JP], wd_d[b, :, 1:JP])
                        nc.sync.dma_start(wu_sb[:, 1:JP], wu_d[b, :, 1:JP])
                    else:
                        nc.gpsimd.dma_start(bias_sb[:], bias_d[b])
                        nc.gpsimd.dma_start(wu_sb[:, 0], wu_d[b, :, 0])
                else:
                    if b == 1 and BUFS == "aggr2":
                        # WAW gate: this poke into b1's wd tile reads b0's
                        # xt0, so every gpsimd DMA (strict FIFO behind this
                        # tile's write) waits ~11us until b0's critical
                        # loads land - b0 gets the full HBM bandwidth for
                        # its working set instead of a ~50% share
                        b0_xt0 = batch_tiles[0][0]
                        nc.gpsimd.tensor_copy(
                            wd_sb[:, 0, 0, 0:1], b0_xt0[:, 0, 0:1]
                        )
                    ld.dma_start(wd_sb[:, 0:1], wd_d[b, :, 0:1])
                    ld.dma_start(xt0_sb[:], xt_d[b, :, 0:2])
                    ld.dma_start(xtr_sb[:], xt_d[b, :, 2:KC])
                    ld.dma_start(bias_sb[:], bias_d[b])
                    ld.dma_start(wu_sb[:, 0], wu_d[b, :, 0])
                # bulk weight loads; PAIR_CHUNK controls granularity
                # (5 = single bulk DMA pair, measured best)
                if b == 0 and BUFS == "aggr2":
                    pass  # bulk already issued on sync above
                else:
                    pc_ = PAIR_CHUNK
                    for jj in range(1, JP, pc_):
                        je = min(jj + pc_, JP)
                        ld.dma_start(wd_sb[:, jj:je], wd_d[b, :, jj:je])
                        ld.dma_start(wu_sb[:, jj:je], wu_d[b, :, jj:je])

            for b in range(BL):
                xt0_sb, xtr_sb, wd_sb, wu_sb, bias_sb = batch_tiles[b]
                for j in range(JP):
                    m0, m1 = 2 * j, 2 * j + 1
                    psum_z = pz_pool.tile([128, S], f32, tag="pz")
                    for k in range(KC):
                        xsrc = xt0_sb[:, k, :] if k < 2 else xtr_sb[:, k - 2, :]
                        nc.tensor.matmul(
                            psum_z[:],
                            lhsT=wd_sb[:, j, k, :],
                            rhs=xsrc,
                            start=(k == 0),
                            stop=(k == KC - 1),
                        )
                    zt_sb = zt_pool.tile([128, S], bf16, tag="zt")
                    nc.scalar.activation(
                        zt_sb[:], psum_z[:], AF.Silu, bias=bias_sb[:, j : j + 1]
                    )
                    u0 = u_pool.tile([128, SC, C], bf16, tag="u0")
                    u1 = u_pool.tile([128, SC, C], bf16, tag="u1")
                    ev = j % 2
                    tail = b == BL - 1 and j >= JP - 2
                    for sc in range(SC):
                        p0 = pu_pool.tile([128, C], f32, tag="pu")
                        p1 = pu_pool.tile([128, C], f32, tag="pu")
                        for cc in range(2):
                            nc.tensor.matmul(
                                p0[:, cc * 512 : (cc + 1) * 512],
                                lhsT=zt_sb[0:64, sc * 128 : (sc + 1) * 128],
                                rhs=wu_sb[0:64, j, cc * 512 : (cc + 1) * 512],
                                start=True,
                                stop=True,
                                tile_position=(0, 0),
                            )
                            nc.tensor.matmul(
                                p1[:, cc * 512 : (cc + 1) * 512],
                                lhsT=zt_sb[64:128, sc * 128 : (sc + 1) * 128],
                                rhs=wu_sb[64:128, j, cc * 512 : (cc + 1) * 512],
                                start=True,
                                stop=True,
                                tile_position=(64, 0),
                            )
                        for pt, ut in ((p0, u0), (p1, u1)):
                            dst = ut[:, sc, :]
                            if tail:
                                # drain the kernel tail twice as fast by
                                # splitting each eviction across engines
                                nc.scalar.copy(dst[:, 0:512], pt[:, 0:512])
                                nc.vector.tensor_copy(dst[:, 512:C], pt[:, 512:C])
                            elif ev % 2 == 0:
                                nc.scalar.copy(dst, pt[:])
                            else:
                                nc.vector.tensor_copy(dst, pt[:])
                            ev += 1
                    if b == BL - 1 and j == JP - 1:
                        for sc in range(SC):
                            nc.sync.dma_start(out_d[m0, b, :, sc], u0[:, sc])
                            nc.sync.dma_start(out_d[m1, b, :, sc], u1[:, sc])
                    else:
                        for half in range(2):
                            hs = slice(half * 2, half * 2 + 2)
                            nc.sync.dma_start(out_d[m0, b, :, hs], u0[:, hs])
                            nc.sync.dma_start(out_d[m1, b, :, hs], u1[:, hs])
    nc.compile()
    return nc


def _build_v3():
    """v2 + fp8e4m3 DoubleRow down-projection.

    The PE streams 1 moving column/cycle in bf16; DoubleRow fp8 streams 2
    (two 128-row k-tiles contracted at once, 0.5 cycles/row).  The down
    GEMM contracts C=1024 = 4 k-tile pairs.  fp8 alone loses too much
    precision (~3.6%), so use 3-term error compensation:
        x @ W  ~=  x_hi@W_hi + x_lo@W_hi + x_hi@W_lo
    (x = x_hi + x_lo exactly to second order; residual x_lo@W_lo ~ 0.1%).
    12 DoubleRow matmuls x 256 cycles replace 8 bf16 matmuls x 512 cycles
    per router pair: down-proj PE time 4096 -> 3072 cycles, -10us/core
    total.  W is pre-scaled by 64 on host (w~N(0,0.01) would land in the
    fp8 denormal range); the silu activation un-scales via scale=1/64.
    DMA bytes are unchanged (hi+lo fp8 = 2 bytes/elem = bf16).
    """
    import concourse.mybir as mybir
    from concourse import bacc, tile

    bf16 = mybir.dt.bfloat16
    fp8 = mybir.dt.float8e4
    f32 = mybir.dt.float32
    AF = mybir.ActivationFunctionType
    DR = mybir.MatmulPerfMode.DoubleRow
    KCP = KC // 2  # k-tile pairs = 4

    nc = bacc.Bacc(
        "TRN2",
        target_bir_lowering=False,
        debug=False,
        num_devices=NCORES,
        num_swdge_queues=4,
    )
    xh_d = nc.declare_dram_parameter("xh", [BL, 128, KCP, 2, S], fp8, isOutput=False)
    xl_d = nc.declare_dram_parameter("xl", [BL, 128, KCP, 2, S], fp8, isOutput=False)
    wh_d = nc.declare_dram_parameter(
        "wh", [BL, 128, JP, KCP, 2, 128], fp8, isOutput=False
    )
    wl_d = nc.declare_dram_parameter(
        "wl", [BL, 128, JP, KCP, 2, 128], fp8, isOutput=False
    )
    wu_d = nc.declare_dram_parameter("wu", [BL, 128, JP, C], bf16, isOutput=False)
    bias_d = nc.declare_dram_parameter("bias", [BL, 128, JP], f32, isOutput=False)
    out_d = nc.declare_dram_parameter("out", [M, BL, 128, SC, C], bf16, isOutput=True)

    with tile.TileContext(nc) as tc:
        with (
            tc.tile_pool(name="xin", bufs=2) as xin_pool,
            tc.tile_pool(name="wpool", bufs=3) as w_pool,
            tc.tile_pool(name="zt", bufs=3) as zt_pool,
            tc.tile_pool(name="usb", bufs=4) as u_pool,
            tc.tile_pool(name="pz", bufs=2, space="PSUM") as pz_pool,
            tc.tile_pool(name="pu", bufs=3, space="PSUM") as pu_pool,
        ):
            for b in range(BL):
                xh0_sb = xin_pool.tile([128, 2, S], fp8, tag="xh0")
                xhr_sb = xin_pool.tile([128, KCP - 1, 2, S], fp8, tag="xhr")
                xl0_sb = xin_pool.tile([128, 2, S], fp8, tag="xl0")
                xlr_sb = xin_pool.tile([128, KCP - 1, 2, S], fp8, tag="xlr")
                wh_sb = w_pool.tile([128, JP, KCP, 2, 128], fp8, tag="wh")
                wl_sb = w_pool.tile([128, JP, KCP, 2, 128], fp8, tag="wl")
                wu_sb = w_pool.tile([128, JP, C], bf16, tag="wu")
                bias_sb = w_pool.tile([128, JP], f32, tag="bias")
                if b == 0:
                    # critical path (term-0 operands) on the sync HWDGE
                    nc.sync.dma_start(wh_sb[:, 0:1], wh_d[b, :, 0:1])
                    nc.sync.dma_start(xh0_sb[:], xh_d[b, :, 0])
                    nc.sync.dma_start(xhr_sb[:], xh_d[b, :, 1:KCP])
                    nc.sync.dma_start(xl0_sb[:], xl_d[b, :, 0])
                    nc.sync.dma_start(xlr_sb[:], xl_d[b, :, 1:KCP])
                    nc.sync.dma_start(wl_sb[:, 0:1], wl_d[b, :, 0:1])
                    nc.gpsimd.dma_start(bias_sb[:], bias_d[b])
                    nc.gpsimd.dma_start(wu_sb[:, 0], wu_d[b, :, 0])
                else:
                    nc.gpsimd.dma_start(wh_sb[:, 0:1], wh_d[b, :, 0:1])
                    nc.gpsimd.dma_start(xh0_sb[:], xh_d[b, :, 0])
                    nc.gpsimd.dma_start(xhr_sb[:], xh_d[b, :, 1:KCP])
                    nc.gpsimd.dma_start(xl0_sb[:], xl_d[b, :, 0])
                    nc.gpsimd.dma_start(xlr_sb[:], xl_d[b, :, 1:KCP])
                    nc.gpsimd.dma_start(bias_sb[:], bias_d[b])
                    nc.gpsimd.dma_start(wu_sb[:, 0], wu_d[b, :, 0])
                    nc.gpsimd.dma_start(wl_sb[:, 0:1], wl_d[b, :, 0:1])
                for jj in range(1, JP):
                    nc.gpsimd.dma_start(wh_sb[:, jj : jj + 1], wh_d[b, :, jj : jj + 1])
                    nc.gpsimd.dma_start(wl_sb[:, jj : jj + 1], wl_d[b, :, jj : jj + 1])
                    nc.gpsimd.dma_start(wu_sb[:, jj], wu_d[b, :, jj])

                for j in range(JP):
                    m0, m1 = 2 * j, 2 * j + 1
                    psum_z = pz_pool.tile([128, S], f32, tag="pz")
                    terms = (
                        (xh0_sb, xhr_sb, wh_sb),
                        (xl0_sb, xlr_sb, wh_sb),
                        (xh0_sb, xhr_sb, wl_sb),
                    )
                    nt = len(terms)
                    for t, (x0, xr, ws) in enumerate(terms):
                        for kp in range(KCP):
                            rhs = x0[:] if kp == 0 else xr[:, kp - 1]
                            nc.tensor.matmul(
                                psum_z[:],
                                lhsT=ws[:, j, kp],
                                rhs=rhs,
                                start=(t == 0 and kp == 0),
                                stop=(t == nt - 1 and kp == KCP - 1),
                                perf_mode=DR,
                            )
                    zt_sb = zt_pool.tile([128, S], bf16, tag="zt")
                    nc.scalar.activation(
                        zt_sb[:],
                        psum_z[:],
                        AF.Silu,
                        bias=bias_sb[:, j : j + 1],
                        scale=1.0 / 64.0,
                    )
                    u0 = u_pool.tile([128, SC, C], bf16, tag="u0")
                    u1 = u_pool.tile([128, SC, C], bf16, tag="u1")
                    ev = j % 2
                    tail = b == BL - 1 and j >= JP - 2
                    for sc in range(SC):
                        p0 = pu_pool.tile([128, C], f32, tag="pu")
                        p1 = pu_pool.tile([128, C], f32, tag="pu")
                        for cc in range(2):
                            nc.tensor.matmul(
                                p0[:, cc * 512 : (cc + 1) * 512],
                                lhsT=zt_sb[0:64, sc * 128 : (sc + 1) * 128],
                                rhs=wu_sb[0:64, j, cc * 512 : (cc + 1) * 512],
                                start=True,
                                stop=True,
                                tile_position=(0, 0),
                            )
                            nc.tensor.matmul(
                                p1[:, cc * 512 : (cc + 1) * 512],
                                lhsT=zt_sb[64:128, sc * 128 : (sc + 1) * 128],
                                rhs=wu_sb[64:128, j, cc * 512 : (cc + 1) * 512],
                                start=True,
                                stop=True,
                                tile_position=(64, 0),
                            )
                        for pt, ut in ((p0, u0), (p1, u1)):
                            dst = ut[:, sc, :]
                            if tail:
                                nc.scalar.copy(dst[:, 0:512], pt[:, 0:512])
                                nc.vector.tensor_copy(dst[:, 512:C], pt[:, 512:C])
                            elif ev % 2 == 0:
                                nc.scalar.copy(dst, pt[:])
                            else:
                                nc.vector.tensor_copy(dst, pt[:])
                            ev += 1
                    if b == BL - 1 and j == JP - 1:
                        for sc in range(SC):
                            nc.sync.dma_start(out_d[m0, b, :, sc], u0[:, sc])
                            nc.sync.dma_start(out_d[m1, b, :, sc], u1[:, sc])
                    else:
                        for half in range(2):
                            hs = slice(half * 2, half * 2 + 2)
                            nc.sync.dma_start(out_d[m0, b, :, hs], u0[:, hs])
                            nc.sync.dma_start(out_d[m1, b, :, hs], u1[:, hs])
    nc.compile()
    return nc


def _build_v4():
    """v2 reworked around the HW-trace finding that the output stream is
    DMA-ISSUE-RATE bound, not bandwidth bound:

    Trace evidence (cold 196.7us / hot 210.6us spans): each DIRECT2D
    issue costs ~0.61us of sequencer time regardless of transfer size
    (128-row descriptor). v2 issues 96 x 512KB output DMAs on the single
    sync HWDGE queue = ~58us of issue time, which caps the output stream
    at ~420GB/s cold and HALF that when the HAM power limiter duty-cycles
    the sequencer clock - the 16 DMA engines starve (busy% drops 82->77)
    and the backlog drains in a 19-32us post-PE tail.

    Changes:
      1. One 1MB DMA per router (48 total): out_d[m, b] is already a
         contiguous [128, SC, C] region. u0's DMA rides the scalar HWDGE
         queue, u1's the sync queue -> ~14.4us of issue per queue, 4x
         slack vs HAM-throttled issue rate.
      2. Evictions pinned: p0 -> u0 always via scalar, p1 -> u1 always
         via vector (same 4+4 balance), so each queue's DMA trigger
         directly follows its own engine's final eviction of that tile.
      3. Startup split across both HWDGE queues: sync carries wd0, xt
         k-chunks, wd[1:2], wd[2:6]; scalar carries bias, wu[0:1],
         wu[1:6]. v2 serialized all 9 b0 issues on sync (~5.5us) and the
         j1 down-proj waited on the trailing 1.25MB wd[1:6] bulk DMA
         (5.5us PE gap at ~10.4us). Predicted: wd1 lands ~11us, PE
         steady from ~13us.
    """
    import concourse.mybir as mybir
    from concourse import bacc, tile

    bf16 = mybir.dt.bfloat16
    f32 = mybir.dt.float32
    AF = mybir.ActivationFunctionType

    nc = bacc.Bacc(
        "TRN2",
        target_bir_lowering=False,
        debug=False,
        num_devices=NCORES,
        num_swdge_queues=4,
    )
    xt_d = nc.declare_dram_parameter("xt", [BL, 128, KC, S], bf16, isOutput=False)
    wd_d = nc.declare_dram_parameter("wd", [BL, 128, JP, KC, 128], bf16, isOutput=False)
    wu_d = nc.declare_dram_parameter("wu", [BL, 128, JP, C], bf16, isOutput=False)
    bias_d = nc.declare_dram_parameter("bias", [BL, 128, JP], f32, isOutput=False)
    out_d = nc.declare_dram_parameter("out", [M, BL, 128, SC, C], bf16, isOutput=True)

    with tile.TileContext(nc) as tc:
        with (
            tc.tile_pool(name="xin", bufs=3) as xin_pool,
            tc.tile_pool(name="wpool", bufs=4) as w_pool,
            tc.tile_pool(name="zt", bufs=2) as zt_pool,
            tc.tile_pool(name="usb", bufs=5) as u_pool,
            tc.tile_pool(name="pz", bufs=2, space="PSUM") as pz_pool,
            tc.tile_pool(name="pu", bufs=3, space="PSUM") as pu_pool,
        ):
            batch_tiles = []
            for b in range(BL):
                xt0_sb = xin_pool.tile([128, 2, S], bf16, tag="xt0")
                xtr_sb = xin_pool.tile([128, KC - 2, S], bf16, tag="xtr")
                wd_sb = w_pool.tile([128, JP, KC, 128], bf16, tag="wd")
                wu_sb = w_pool.tile([128, JP, C], bf16, tag="wu")
                bias_sb = w_pool.tile([128, JP], f32, tag="bias")
                batch_tiles.append((xt0_sb, xtr_sb, wd_sb, wu_sb, bias_sb))
                if b == 0:
                    # b0 critical path split across BOTH HWDGE queues
                    nc.sync.dma_start(wd_sb[:, 0:1], wd_d[b, :, 0:1])
                    nc.sync.dma_start(xt0_sb[:], xt_d[b, :, 0:2])
                    nc.scalar.dma_start(bias_sb[:], bias_d[b])
                    nc.scalar.dma_start(wu_sb[:, 0:1], wu_d[b, :, 0:1])
                    for kk in range(0, KC - 2, 2):
                        nc.sync.dma_start(
                            xtr_sb[:, kk : kk + 2], xt_d[b, :, kk + 2 : kk + 4]
                        )
                    nc.sync.dma_start(wd_sb[:, 1:2], wd_d[b, :, 1:2])
                    nc.scalar.dma_start(wu_sb[:, 1:JP], wu_d[b, :, 1:JP])
                    nc.sync.dma_start(wd_sb[:, 2:JP], wd_d[b, :, 2:JP])
                else:
                    if b == 1:
                        # WAW gate: this poke into b1's wd tile reads b0's
                        # xt0, so every gpsimd DMA (strict FIFO behind this
                        # tile's write) waits until b0's critical loads
                        # land - b0 gets full HBM bandwidth for its
                        # working set instead of a ~50% share
                        b0_xt0 = batch_tiles[0][0]
                        nc.gpsimd.tensor_copy(wd_sb[:, 0, 0, 0:1], b0_xt0[:, 0, 0:1])
                    nc.gpsimd.dma_start(wd_sb[:, 0:1], wd_d[b, :, 0:1])
                    nc.gpsimd.dma_start(xt0_sb[:], xt_d[b, :, 0:2])
                    nc.gpsimd.dma_start(xtr_sb[:], xt_d[b, :, 2:KC])
                    nc.gpsimd.dma_start(bias_sb[:], bias_d[b])
                    nc.gpsimd.dma_start(wu_sb[:, 0], wu_d[b, :, 0])
                    nc.gpsimd.dma_start(wd_sb[:, 1:JP], wd_d[b, :, 1:JP])
                    nc.gpsimd.dma_start(wu_sb[:, 1:JP], wu_d[b, :, 1:JP])

            for b in range(BL):
                xt0_sb, xtr_sb, wd_sb, wu_sb, bias_sb = batch_tiles[b]
                for j in range(JP):
                    m0, m1 = 2 * j, 2 * j + 1
                    psum_z = pz_pool.tile([128, S], f32, tag="pz")
                    for k in range(KC):
                        xsrc = xt0_sb[:, k, :] if k < 2 else xtr_sb[:, k - 2, :]
                        nc.tensor.matmul(
                            psum_z[:],
                            lhsT=wd_sb[:, j, k, :],
                            rhs=xsrc,
                            start=(k == 0),
                            stop=(k == KC - 1),
                        )
                    zt_sb = zt_pool.tile([128, S], bf16, tag="zt")
                    nc.scalar.activation(
                        zt_sb[:], psum_z[:], AF.Silu, bias=bias_sb[:, j : j + 1]
                    )
                    u0 = u_pool.tile([128, SC, C], bf16, tag="u0")
                    u1 = u_pool.tile([128, SC, C], bf16, tag="u1")
                    for sc in range(SC):
                        p0 = pu_pool.tile([128, C], f32, tag="pu")
                        p1 = pu_pool.tile([128, C], f32, tag="pu")
                        for cc in range(2):
                            nc.tensor.matmul(
                                p0[:, cc * 512 : (cc + 1) * 512],
                                lhsT=zt_sb[0:64, sc * 128 : (sc + 1) * 128],
                                rhs=wu_sb[0:64, j, cc * 512 : (cc + 1) * 512],
                                start=True,
                                stop=True,
                                tile_position=(0, 0),
                            )
                            nc.tensor.matmul(
                                p1[:, cc * 512 : (cc + 1) * 512],
                                lhsT=zt_sb[64:128, sc * 128 : (sc + 1) * 128],
                                rhs=wu_sb[64:128, j, cc * 512 : (cc + 1) * 512],
                                start=True,
                                stop=True,
                                tile_position=(64, 0),
                            )
                        # pinned: scalar always evicts p0->u0, vector p1->u1,
                        # so each output queue's DMA follows its own engine
                        nc.scalar.copy(u0[:, sc, :], p0[:])
                        nc.vector.tensor_copy(u1[:, sc, :], p1[:])
                    nc.scalar.dma_start(out_d[m0, b], u0[:])
                    nc.sync.dma_start(out_d[m1, b], u1[:])
    nc.compile()
    return nc


def _build_v5(gp_out=False, split_w=False, late_gate=False):
    """v2 with EXACTLY one change: per-router 1MB output DMAs (48 on sync)
    instead of per-half 512KB ones (96) mid-kernel; tail unchanged.

    Rationale from the HW trace: each DIRECT2D issue costs ~0.61us of
    sync-sequencer time regardless of size (128-row descriptors), so v2
    spends ~58us issuing outputs - at the HAM-throttled (half-clock)
    issue rate that caps the output stream below HBM bandwidth and the
    backlog drains in a 19-32us post-PE tail. 48 issues halve that.
    (v4's further step - u0 DMAs on the scalar queue + pinned evictions +
    dual-queue startup - regressed to 252us: scalar-queue output DMAs
    stall the scalar engine's silu/eviction stream and the dual-queue
    startup scrambles b0's load ordering; avoided here.)

    gp_out: route u0's output DMA to the gpsimd SWDGE queue for b>=2
    (inputs are done by ~53us) plus the last pair's per-sc u0 DMAs -
    second issue engine for the output stream.

    split_w: tile dependencies are tracked per-TILE, not per-region: in
    the v5 trace the first matmul waited until b0's BULK wd[1:JP] DMA
    completed (15.06us) because wd_sb was one tile written by two DMAs.
    Splitting wd/wu into {j0}, {j1}, {j2:} tiles lets pair 0 start at
    ~9us and pairs 1+ stream in arrival order.
    """
    import concourse.mybir as mybir
    from concourse import bacc, tile

    bf16 = mybir.dt.bfloat16
    f32 = mybir.dt.float32
    AF = mybir.ActivationFunctionType

    nc = bacc.Bacc(
        "TRN2",
        target_bir_lowering=False,
        debug=False,
        num_devices=NCORES,
        num_swdge_queues=4,
    )
    xt_d = nc.declare_dram_parameter("xt", [BL, 128, KC, S], bf16, isOutput=False)
    wd_d = nc.declare_dram_parameter("wd", [BL, 128, JP, KC, 128], bf16, isOutput=False)
    wu_d = nc.declare_dram_parameter("wu", [BL, 128, JP, C], bf16, isOutput=False)
    bias_d = nc.declare_dram_parameter("bias", [BL, 128, JP], f32, isOutput=False)
    out_d = nc.declare_dram_parameter("out", [M, BL, 128, SC, C], bf16, isOutput=True)

    with tile.TileContext(nc) as tc:
        xin_b, w_b, zt_b, u_b = 3, 4, 2, 5
        with (
            tc.tile_pool(name="xin", bufs=xin_b) as xin_pool,
            tc.tile_pool(name="wpool", bufs=w_b) as w_pool,
            tc.tile_pool(name="zt", bufs=zt_b) as zt_pool,
            tc.tile_pool(name="usb", bufs=u_b) as u_pool,
            tc.tile_pool(name="pz", bufs=2, space="PSUM") as pz_pool,
            tc.tile_pool(name="pu", bufs=3, space="PSUM") as pu_pool,
        ):
            batch_tiles = []
            for b in range(BL):
                xt0_sb = xin_pool.tile([128, 2, S], bf16, tag="xt0")
                xtr_sb = xin_pool.tile([128, KC - 2, S], bf16, tag="xtr")
                if split_w:
                    # separate tiles per dependency chunk: readers of tile
                    # regions wait on ALL the tile's writers, so j0/j1 must
                    # not share a tile with the bulk DMA
                    wd_sb = (
                        w_pool.tile([128, 1, KC, 128], bf16, tag="wd0", name="wd0"),
                        w_pool.tile([128, 1, KC, 128], bf16, tag="wd1", name="wd1"),
                        w_pool.tile(
                            [128, JP - 2, KC, 128], bf16, tag="wdr", name="wdr"
                        ),
                    )
                    wu_sb = (
                        w_pool.tile([128, 1, C], bf16, tag="wu0", name="wu0"),
                        w_pool.tile([128, 1, C], bf16, tag="wu1", name="wu1"),
                        w_pool.tile([128, JP - 2, C], bf16, tag="wur", name="wur"),
                    )
                else:
                    wd_sb = (
                        w_pool.tile([128, JP, KC, 128], bf16, tag="wd", name="wd"),
                    )
                    wu_sb = (w_pool.tile([128, JP, C], bf16, tag="wu", name="wu"),)
                bias_sb = w_pool.tile([128, JP], f32, tag="bias")
                batch_tiles.append((xt0_sb, xtr_sb, wd_sb, wu_sb, bias_sb))

                eng = nc.sync if b == 0 else nc.gpsimd
                if b == 1:
                    # WAW gate: poke into b1's first wd tile reads a b0 tile
                    # so all gpsimd bulk DMAs queue behind b0's critical path.
                    # late_gate reads b0's wu1 (the LAST of b0's j0+j1
                    # working-set DMAs, sync #9) instead of xt0 (#2): the v7
                    # trace shows the 12.75MB b1-b3 flood starting at xt0
                    # arrival (~8.3us) starves b0's remaining critical loads
                    # (down matmuls paced 1.4-2.4us apart to ~23us); gating
                    # on wu1 lets them land at full bandwidth by ~11.4us.
                    if late_gate and split_w:
                        b0_gate = batch_tiles[0][3][1][:, 0, 0:1]
                    else:
                        b0_gate = batch_tiles[0][0][:, 0, 0:1]
                    nc.gpsimd.tensor_copy(wd_sb[0][:, 0, 0, 0:1], b0_gate)
                if split_w:
                    eng.dma_start(wd_sb[0][:, 0], wd_d[b, :, 0])
                    eng.dma_start(xt0_sb[:], xt_d[b, :, 0:2])
                    if b == 0:
                        for kk in range(0, KC - 2, 2):
                            eng.dma_start(
                                xtr_sb[:, kk : kk + 2], xt_d[b, :, kk + 2 : kk + 4]
                            )
                    else:
                        eng.dma_start(xtr_sb[:], xt_d[b, :, 2:KC])
                    eng.dma_start(bias_sb[:], bias_d[b])
                    eng.dma_start(wu_sb[0][:, 0], wu_d[b, :, 0])
                    eng.dma_start(wd_sb[1][:, 0], wd_d[b, :, 1])
                    eng.dma_start(wu_sb[1][:, 0], wu_d[b, :, 1])
                    eng.dma_start(wd_sb[2][:], wd_d[b, :, 2:JP])
                    eng.dma_start(wu_sb[2][:], wu_d[b, :, 2:JP])
                else:
                    eng.dma_start(wd_sb[0][:, 0:1], wd_d[b, :, 0:1])
                    eng.dma_start(xt0_sb[:], xt_d[b, :, 0:2])
                    if b == 0:
                        for kk in range(0, KC - 2, 2):
                            eng.dma_start(
                                xtr_sb[:, kk : kk + 2], xt_d[b, :, kk + 2 : kk + 4]
                            )
                    else:
                        eng.dma_start(xtr_sb[:], xt_d[b, :, 2:KC])
                    eng.dma_start(bias_sb[:], bias_d[b])
                    eng.dma_start(wu_sb[0][:, 0], wu_d[b, :, 0])
                    eng.dma_start(wd_sb[0][:, 1:JP], wd_d[b, :, 1:JP])
                    eng.dma_start(wu_sb[0][:, 1:JP], wu_d[b, :, 1:JP])

            def wd_at(wd_sb, j):
                if not split_w:
                    return wd_sb[0][:, j]
                return wd_sb[min(j, 2)][:, 0 if j < 2 else j - 2]

            def wu_at(wu_sb, j):
                if not split_w:
                    return wu_sb[0][:, j]
                return wu_sb[min(j, 2)][:, 0 if j < 2 else j - 2]

            for b in range(BL):
                xt0_sb, xtr_sb, wd_sb, wu_sb, bias_sb = batch_tiles[b]
                for j in range(JP):
                    m0, m1 = 2 * j, 2 * j + 1
                    wd_j = wd_at(wd_sb, j)
                    wu_j = wu_at(wu_sb, j)
                    psum_z = pz_pool.tile([128, S], f32, tag="pz")
                    for k in range(KC):
                        xsrc = xt0_sb[:, k, :] if k < 2 else xtr_sb[:, k - 2, :]
                        nc.tensor.matmul(
                            psum_z[:],
                            lhsT=wd_j[:, k, :],
                            rhs=xsrc,
                            start=(k == 0),
                            stop=(k == KC - 1),
                        )
                    zt_sb = zt_pool.tile([128, S], bf16, tag="zt")
                    nc.scalar.activation(
                        zt_sb[:], psum_z[:], AF.Silu, bias=bias_sb[:, j : j + 1]
                    )
                    u0 = u_pool.tile([128, SC, C], bf16, tag="u0")
                    u1 = u_pool.tile([128, SC, C], bf16, tag="u1")
                    ev = j % 2
                    tail = b == BL - 1 and j >= JP - 2
                    for sc in range(SC):
                        p0 = pu_pool.tile([128, C], f32, tag="pu")
                        p1 = pu_pool.tile([128, C], f32, tag="pu")
                        for cc in range(2):
                            nc.tensor.matmul(
                                p0[:, cc * 512 : (cc + 1) * 512],
                                lhsT=zt_sb[0:64, sc * 128 : (sc + 1) * 128],
                                rhs=wu_j[0:64, cc * 512 : (cc + 1) * 512],
                                start=True,
                                stop=True,
                                tile_position=(0, 0),
                            )
                            nc.tensor.matmul(
                                p1[:, cc * 512 : (cc + 1) * 512],
                                lhsT=zt_sb[64:128, sc * 128 : (sc + 1) * 128],
                                rhs=wu_j[64:128, cc * 512 : (cc + 1) * 512],
                                start=True,
                                stop=True,
                                tile_position=(64, 0),
                            )
                        for pt, ut in ((p0, u0), (p1, u1)):
                            dst = ut[:, sc, :]
                            if tail:
                                nc.scalar.copy(dst[:, 0:512], pt[:, 0:512])
                                nc.vector.tensor_copy(dst[:, 512:C], pt[:, 512:C])
                            elif ev % 2 == 0:
                                nc.scalar.copy(dst, pt[:])
                            else:
                                nc.vector.tensor_copy(dst, pt[:])
                            ev += 1
                    if b == BL - 1 and j == JP - 1:
                        for sc in range(SC):
                            if gp_out:
                                nc.gpsimd.dma_start(out_d[m0, b, :, sc], u0[:, sc])
                            else:
                                nc.sync.dma_start(out_d[m0, b, :, sc], u0[:, sc])
                            nc.sync.dma_start(out_d[m1, b, :, sc], u1[:, sc])
                    else:
                        # from b>=2 the gpsimd SWDGE queue has finished all
                        # input loads; routing u0's DMA there gives a second
                        # issue engine so sync's in-stream eviction waits no
                        # longer starve the 16 DMA engines (88-90% busy in
                        # the v5 trace, and the missing ~10% is the tail)
                        if gp_out and b >= 2:
                            nc.gpsimd.dma_start(out_d[m0, b], u0[:])
                        else:
                            nc.sync.dma_start(out_d[m0, b], u0[:])
                        nc.sync.dma_start(out_d[m1, b], u1[:])
    nc.compile()
    return nc


_BUILDERS = {
    3: _build_v3,
    4: _build_v4,
    5: _build_v5,
    6: lambda: _build_v5(gp_out=True),
    7: lambda: _build_v5(split_w=True),
    8: lambda: _build_v5(gp_out=True, split_w=True),
    9: lambda: _build_v5(split_w=True, late_gate=True),
    10: lambda: _build_v5(gp_out=True, split_w=True, late_gate=True),
}


def _get_nc(variant=0):
    if variant not in _nc_cache:
        if variant in _BUILDERS:
            _nc_cache[variant] = _BUILDERS[variant]()
        else:
            _nc_cache[variant] = _build(variant)
    return _nc_cache[variant]


def kernel(x, expert_index, down_w, down_b, up_w):
    global last_results
    from concourse.bass_utils import run_bass_kernel_spmd

    x = np.asarray(x, dtype=np.float32)              # [B, S, C]
    idx = np.asarray(expert_index).astype(np.int64)  # [M, B]
    down_w = np.asarray(down_w, dtype=np.float32)    # [M, N, C, D]
    down_b = np.asarray(down_b, dtype=np.float32)    # [M, N, D]
    up_w = np.asarray(up_w, dtype=np.float32)        # [M, N, D, C]

    m_idx = np.arange(M)[:, None]
    wd_g = down_w[m_idx, idx]                        # [M, B, C, D]
    bb_g = down_b[m_idx, idx]                        # [M, B, D]
    wu_g = up_w[m_idx, idx]                          # [M, B, D, C]

    variant = int(os.environ.get("KERNEL_VARIANT", "5"))

    # xt[b, p, k, s] = x[b, s, k*128+p]
    xt_f = np.ascontiguousarray(
        x.transpose(0, 2, 1).reshape(B, KC, 128, S).transpose(0, 2, 1, 3)
    )
    # wd[b, p, j, k, dd]: dd in [0,128) holds router 2j (d=dd) in the low
    # 64 columns and router 2j+1 (d=dd-64) in the high 64 columns, so one
    # [128,128] stationary load covers the pair
    wd_f = np.ascontiguousarray(
        wd_g.reshape(JP, 2, B, KC, 128, D)
        .transpose(2, 4, 0, 3, 1, 5)
        .reshape(B, 128, JP, KC, 128)
    )
    # wu[b, p, j, c]: partitions 0-63 hold router 2j (d = p), partitions
    # 64-127 hold router 2j+1 (d = p-64)
    wu_p = wu_g.reshape(JP, 2, B, D, C).transpose(2, 1, 3, 0, 4)  # [B,2,D,JP,C]
    wu = np.ascontiguousarray(wu_p.reshape(B, 128, JP, C)).astype(BF16)
    # bias[b, p, j], same partition packing as wu
    bias_p = bb_g.reshape(JP, 2, B, D).transpose(2, 1, 3, 0)      # [B,2,D,JP]
    bias = np.ascontiguousarray(bias_p.reshape(B, 128, JP)).astype(np.float32)

    if variant == 3:
        F8 = ml_dtypes.float8_e4m3
        KCP = KC // 2
        xh = xt_f.astype(F8)
        xl = (xt_f - xh.astype(np.float32)).astype(F8)
        xh = xh.reshape(B, 128, KCP, 2, S)
        xl = xl.reshape(B, 128, KCP, 2, S)
        wds = wd_f * 64.0  # w ~ N(0, 0.01) sits in fp8 denormal range unscaled
        wh = wds.astype(F8)
        wl = (wds - wh.astype(np.float32)).astype(F8)
        wh = wh.reshape(B, 128, JP, KCP, 2, 128)
        wl = wl.reshape(B, 128, JP, KCP, 2, 128)
        per_core = {"xh": xh, "xl": xl, "wh": wh, "wl": wl, "wu": wu, "bias": bias}
    else:
        per_core = {
            "xt": xt_f.astype(BF16),
            "wd": wd_f.astype(BF16),
            "wu": wu,
            "bias": bias,
        }

    in_maps = []
    for core in range(NCORES):
        sl = slice(core * BL, (core + 1) * BL)
        in_maps.append({k: v[sl] for k, v in per_core.items()})

    nc = _get_nc(variant)
    trace_kwargs = {}
    if os.environ.get("KERNEL_TRACE_ALL"):
        trace_kwargs["trace_cores"] = list(range(NCORES))
    res = None
    for attempt in range(3):
        try:
            res = run_bass_kernel_spmd(
                nc, in_maps, core_ids=list(range(NCORES)), trace=TRACE, **trace_kwargs
            )
            break
        except Exception:
            # transient NRT_EXEC_UNIT_UNRECOVERABLE has been observed on a
            # process's first execute (stale device state from a prior
            # process); give the runtime a moment to recover, then retry
            if attempt == 2:
                raise
            time.sleep(10.0 * (attempt + 1))
    last_results = res

    out = np.empty((M, B, S, C), dtype=np.float32)
    for core in range(NCORES):
        sl = slice(core * BL, (core + 1) * BL)
        # dev out [M, BL, p, sc, c] -> [M, BL, s = sc*128+p, c]
        dev = res.results[core]["out"]
        out[:, sl] = dev.transpose(0, 1, 3, 2, 4).reshape(M, BL, S, C).astype(
            np.float32
        )
    return out

